# revision 1
# baseline (speedup 1.0000x reference)
"""TRN2 Bass kernel for nn_BlendEmoBackbone: gated audio mixer + low-rank
multiplicative fusion, data-parallel over batch on 8 NeuronCores.

Strategy:
- Pure data parallel: each core handles B/8 = 512 batch rows; the gate MLP
  weights and the LMF factor tensors are replicated.
- All activations kept in transposed [feature, batch] layout on-chip so every
  matmul contracts over the partition dim; tokens are transposed on the host.
- LayerNorms over the feature dim use PE ones-matmuls for partition sums,
  with the -mu term folded into gate matmuls as an extra contraction row.
- The LMF where(mask, z, 1) and the x_aug ones-column are folded into the
  factor matmul as a K=2 tail tile ([bias_row; ones_row] x [mask; 1-mask]).
- rank_w is folded into the audio factor slices on the host.
- Matmuls run in float32r (TF32-like, full PE rate for N>=256).
"""

import numpy as np
from contextlib import ExitStack

import concourse.bass as bass
from concourse import bacc
import concourse.tile as tile
from concourse import mybir
from concourse.bass_utils import run_bass_kernel_spmd

B, M, H, R = 4096, 4, 1024, 10
NCORES = 8
BS = B // NCORES          # 512 batch rows per core
MID = 512
P = 128
HT = H // P               # 8 h-tiles
MT = MID // P             # 4 mid-tiles
D3 = 3 * H
OTHERS = (0, 2, 3)
AUDIO = 1
EPS = 1e-5

f32 = mybir.dt.float32
f32r = mybir.dt.float32r
u8 = mybir.dt.uint8
AF = mybir.ActivationFunctionType
OP = mybir.AluOpType

TRACE = False
LAST_RESULTS = None

_cached_nc = None


def _build():
    nc = bacc.Bacc("TRN2", target_bir_lowering=False, debug=False)

    # ---- DRAM parameters (per core) ----
    tokT = nc.declare_dram_parameter("tokT", [M, H, BS], f32r, isOutput=False)
    # u8 rows: 0-2 pv_j, 3-5 mo_j, 6 am(aum), 7 ma
    u8rows = nc.declare_dram_parameter("u8rows", [8, BS], u8, isOutput=False)
    cmrows = nc.declare_dram_parameter("cmrows", [3, BS], f32, isOutput=False)
    uv = nc.declare_dram_parameter("uv", [M, 2, BS], f32r, isOutput=False)
    WGO = nc.declare_dram_parameter("WGO", [3 * HT, P, MID], f32r, isOutput=False)
    WGA = nc.declare_dram_parameter("WGA", [3 * HT, P, MID], f32r, isOutput=False)
    WGOe = nc.declare_dram_parameter("WGOe", [1, MID], f32r, isOutput=False)
    WGAe = nc.declare_dram_parameter("WGAe", [1, MID], f32r, isOutput=False)
    W2 = nc.declare_dram_parameter("W2", [P, MT, 2], f32r, isOutput=False)
    CB = nc.declare_dram_parameter("CB", [P, 8], f32, isOutput=False)
    SC = nc.declare_dram_parameter("SC", [1, 8], f32, isOutput=False)
    # [ht_out, kt, P, P] tiled weight blocks (lhsT layout)
    A2OT = nc.declare_dram_parameter("A2OT", [HT, HT, P, P], f32r, isOutput=False)
    O2AT = nc.declare_dram_parameter("O2AT", [HT, HT, P, P], f32r, isOutput=False)
    OUTWT = nc.declare_dram_parameter("OUTWT", [HT, HT, P, P], f32r, isOutput=False)
    # cols: ln_o_w 0:8, ln_o_b 8:16, ln_a_w 16:24, ln_a_b 24:32,
    #       ln1w 32:40, ln1b 40:48, ln2w 48:56, ln2b 56:64, outb 64:72, lmfb 72:80
    LNV = nc.declare_dram_parameter("LNV", [P, 80], f32, isOutput=False)
    # FT[..., 8, 0:2, :] = [bias_row; ones_or_rankw_row], rest of block 8 unused
    FT = nc.declare_dram_parameter("FT", [R, HT, M, 9, P, P], f32r, isOutput=False)
    KON = nc.declare_dram_parameter("KON", [P, 1], f32r, isOutput=False)
    OUT = nc.declare_dram_parameter("outT", [H, BS], f32, isOutput=True)

    with tile.TileContext(nc) as tc, ExitStack() as ctx:
        kp = ctx.enter_context(tc.tile_pool(name="konst", bufs=1))
        tokp = ctx.enter_context(tc.tile_pool(name="tokp", bufs=1))
        big = ctx.enter_context(tc.tile_pool(name="big", bufs=1))
        wk = ctx.enter_context(tc.tile_pool(name="wk", bufs=2))
        bcp = ctx.enter_context(tc.tile_pool(name="bcp", bufs=1))
        sqp = ctx.enter_context(tc.tile_pool(name="sqp", bufs=2))
        wgp = ctx.enter_context(tc.tile_pool(name="wgp", bufs=2))
        ftp = ctx.enter_context(tc.tile_pool(name="ftp", bufs=2))
        rowp = ctx.enter_context(tc.tile_pool(name="rowp", bufs=1))
        ppz = ctx.enter_context(tc.tile_pool(name="ppz", bufs=4, space="PSUM"))
        pps = ctx.enter_context(tc.tile_pool(name="pps", bufs=1, space="PSUM"))
        ppo = ctx.enter_context(tc.tile_pool(name="ppo", bufs=2, space="PSUM"))

        # ---- constants / small loads ----
        ones_k = kp.tile([P, 1], f32r)
        nc.sync.dma_start(out=ones_k, in_=KON.ap())
        ones1 = kp.tile([1, P], f32)
        nc.vector.memset(ones1, 1.0)

        def bc_row_dma(dst, src_ap):
            nc.sync.dma_start(
                out=dst,
                in_=bass.AP(
                    tensor=src_ap.tensor, offset=src_ap.offset, ap=[[0, P], [1, BS]]
                ),
            )

        u8t = []
        for i in range(8):
            t = kp.tile([P, BS], u8, tag=f"u8_{i}")
            bc_row_dma(t, u8rows.ap()[i : i + 1, :])
            u8t.append(t)
        pv_t, mo_t, am_t, ma_t = u8t[0:3], u8t[3:6], u8t[6], u8t[7]
        cm_t = []
        for i in range(3):
            t = kp.tile([P, BS], f32, tag=f"cm_{i}")
            bc_row_dma(t, cmrows.ap()[i : i + 1, :])
            cm_t.append(t)
        uvt = []
        for m in range(M):
            t = kp.tile([2, BS], f32r, tag=f"uv_{m}")
            nc.sync.dma_start(out=t, in_=uv.ap()[m])
            uvt.append(t)
        cbt = kp.tile([P, 8], f32)
        nc.sync.dma_start(out=cbt, in_=CB.ap())
        sct = kp.tile([1, 8], f32)
        nc.sync.dma_start(out=sct, in_=SC.ap())
        lnv = kp.tile([P, 80], f32)
        nc.sync.dma_start(out=lnv, in_=LNV.ap())
        w2t = kp.tile([P, MT, 2], f32r)
        nc.sync.dma_start(out=w2t, in_=W2.ap())

        # ---- tokens (transposed) ----
        tok = tokp.tile([P, M, HT, BS], f32r)
        for m in range(M):
            src = tokT.ap()[m].rearrange("(ht p) b -> p ht b", p=P)
            for ht in range(HT):
                nc.sync.dma_start(out=tok[:, m, ht, :], in_=src[:, ht, :])

        def tk(m, kt):
            return tok[:, m, kt, :]

        # ---- helpers ----
        def ln_rows(stat, n, tag):
            """From psum stat banks (sum, sumsq) compute negmu [1,BS] f32r
            and rinv [1,BS] f32 rows."""
            statA, statB = stat
            negmu = rowp.tile([1, BS], f32r, tag="negmu", name=f"negmu_{tag}")
            nc.scalar.activation(negmu, statA[0:1, :], AF.Copy, bias=0.0, scale=-1.0 / n)
            ex2 = rowp.tile([1, BS], f32, tag="ex2", name=f"ex2_{tag}")
            nc.scalar.activation(ex2, statB[0:1, :], AF.Copy, bias=0.0, scale=1.0 / n)
            msq = rowp.tile([1, BS], f32, tag="msq", name=f"msq_{tag}")
            nc.scalar.activation(msq, negmu, AF.Square)
            nc.vector.tensor_sub(ex2, ex2, msq)                      # var in place
            nc.scalar.activation(msq, ex2, AF.Sqrt, bias=sct[0:1, 2:3], scale=1.0)  # sd
            rinv = rowp.tile([1, BS], f32, tag="rinv", name=f"rinv_{tag}")
            nc.vector.reciprocal(rinv, msq)
            return negmu, rinv

        def bcast(row, tag):
            """Broadcast a [1,BS] f32 row to a [P,BS] f32 sbuf tile via PE outer."""
            po = ppo.tile([P, BS], f32, tag="outer")
            nc.tensor.matmul(po, ones1, row, start=True, stop=True)
            sb = bcp.tile([P, BS], f32, tag=f"bc_{tag}")
            nc.vector.tensor_copy(sb, po)
            return sb

        def colsum_stats(stat, pairs):
            """Accumulate sum (bank A) and sumsq (bank B) over the given
            (tile, square_tile) pairs of [P,BS] f32r tiles."""
            statA, statB = stat
            n = len(pairs)
            for i, (t, sq) in enumerate(pairs):
                nc.tensor.matmul(statA[0:1, :], ones_k, t, start=(i == 0), stop=(i == n - 1))
                nc.tensor.matmul(statB[0:1, :], ones_k, sq, start=(i == 0), stop=(i == n - 1))

        # ---- a2o = audio @ a2o_w.T, in T layout [H, BS] ----
        a2or = big.tile([P, HT, BS], f32, tag="axr")
        for ho in range(HT):
            ps = ppz.tile([P, BS], f32, tag="z")
            for kt in range(HT):
                wt = wgp.tile([P, P], f32r, tag="ww")
                nc.sync.dma_start(out=wt, in_=A2OT.ap()[ho, kt])
                nc.tensor.matmul(ps, wt, tk(AUDIO, kt), start=(kt == 0), stop=(kt == HT - 1))
            nc.vector.tensor_copy(a2or[:, ho, :], ps)

        omt = big.tile([P, HT, BS], f32r, tag="om")  # others_mean accumulator
        mix_src = {"x": a2or}  # a2o for others-gates, o2a for the audio gate

        def gate_and_mix(j, mj):
            """j: 0..2 index into OTHERS, or 3 for the audio gate."""
            is_audio = j == 3
            t_m = AUDIO if is_audio else mj

            def s_tile(kt):
                return omt[:, kt, :] if is_audio else tk(AUDIO, kt)

            # |t-s| tiles + squares + LN stats over the 3H concat features
            stat = (pps.tile([1, BS], f32, tag="statA", name="statA"),
            pps.tile([1, BS], f32, tag="statB", name="statB"))
            abs_t = big.tile([P, HT, BS], f32r, tag="abs")
            pairs = []
            for kt in range(HT):
                d = wk.tile([P, BS], f32, tag="d")
                nc.vector.tensor_sub(d, tk(t_m, kt), s_tile(kt))
                nc.scalar.activation(abs_t[:, kt, :], d, AF.Abs)
                sqd = sqp.tile([P, BS], f32r, tag="sq")
                nc.vector.tensor_mul(sqd, d, d)
                pairs.append((abs_t[:, kt, :], sqd))
                sqt = sqp.tile([P, BS], f32r, tag="sq")
                nc.vector.tensor_mul(sqt, tk(t_m, kt), tk(t_m, kt))
                pairs.append((tk(t_m, kt), sqt))
                sqs = sqp.tile([P, BS], f32r, tag="sq")
                nc.vector.tensor_mul(sqs, s_tile(kt), s_tile(kt))
                pairs.append((s_tile(kt), sqs))
            colsum_stats(stat, pairs)
            negmu, rinv = ln_rows(stat, D3, "g")

            # gate layer 1: psum[mt] = W~.T @ [t; s; |t-s|] - mu*c1
            WG = WGA if is_audio else WGO
            wge = rowp.tile([1, MID], f32r, tag="wge", name=f"wge{j}")
            nc.sync.dma_start(out=wge, in_=(WGAe if is_audio else WGOe).ap())
            gps = [ppz.tile([P, BS], f32, tag="z", name=f"gps{mt}") for mt in range(MT)]
            for kt in range(3 * HT):
                wt = wgp.tile([P, MID], f32r, tag="wg")
                nc.sync.dma_start(out=wt, in_=WG.ap()[kt])
                part, k = kt // HT, kt % HT
                rhs = tk(t_m, k) if part == 0 else (s_tile(k) if part == 1 else abs_t[:, k, :])
                for mt in range(MT):
                    nc.tensor.matmul(
                        gps[mt], wt[:, mt * P : (mt + 1) * P], rhs,
                        start=(kt == 0), stop=False,
                    )
            for mt in range(MT):
                nc.tensor.matmul(
                    gps[mt], wge[0:1, mt * P : (mt + 1) * P], negmu,
                    start=False, stop=True,
                )
            rb = bcast(rinv, "rb")
            cb_off = 4 if is_audio else 0
            col = 1 if is_audio else 0
            gp = pps.tile([1, BS], f32, tag="statA", name="gp")
            for mt in range(MT):
                hm = wk.tile([P, BS], f32, tag="hm")
                nc.vector.tensor_mul(hm, gps[mt], rb)
                hg1 = wk.tile([P, BS], f32r, tag="hg", name=f"hg{mt}")
                nc.scalar.activation(
                    hg1, hm, AF.Gelu,
                    bias=cbt[:, cb_off + mt : cb_off + mt + 1], scale=1.0,
                )
                nc.tensor.matmul(
                    gp[:, :], w2t[:, mt, col : col + 1], hg1,
                    start=(mt == 0), stop=(mt == MT - 1),
                )
            g_row = rowp.tile([1, BS], f32, tag="g_row")
            nc.scalar.activation(
                g_row, gp[:, :], AF.Sigmoid,
                bias=sct[0:1, col : col + 1], scale=1.0,
            )
            gb = bcast(g_row, "gb")

            # pre = t + g * (a2o | o2a); LN over H; blend into tok in place
            src = mix_src["x"]
            pre = big.tile([P, HT, BS], f32r, tag="abs", name="pre")
            stat2 = (pps.tile([1, BS], f32, tag="statA", name="stat2A"),
            pps.tile([1, BS], f32, tag="statB", name="stat2B"))
            pairs2 = []
            for kt in range(HT):
                tmp = wk.tile([P, BS], f32, tag="hm")
                nc.vector.tensor_mul(tmp, gb, src[:, kt, :])
                nc.vector.tensor_add(pre[:, kt, :], tmp, tk(t_m, kt))
                sq = sqp.tile([P, BS], f32r, tag="sq")
                nc.vector.tensor_mul(sq, pre[:, kt, :], pre[:, kt, :])
                pairs2.append((pre[:, kt, :], sq))
            colsum_stats(stat2, pairs2)
            negmu2, rinv2 = ln_rows(stat2, H, "u")
            mb = bcast(negmu2.bitcast(f32), "mb")
            rb2 = bcast(rinv2, "rb2")
            wcol = 16 if is_audio else 0
            bcol = 24 if is_audio else 8
            for kt in range(HT):
                nc.vector.tensor_add(pre[:, kt, :], pre[:, kt, :], mb)
                nc.vector.tensor_mul(pre[:, kt, :], pre[:, kt, :], rb2)
                nc.vector.tensor_scalar(
                    pre[:, kt, :], pre[:, kt, :],
                    lnv[:, wcol + kt : wcol + kt + 1], lnv[:, bcol + kt : bcol + kt + 1],
                    op0=OP.mult, op1=OP.add,
                )
                # blend = big_mask*t + small_mask*(upd - t), in place into tok
                bm = ma_t if is_audio else mo_t[j]
                sm = am_t if is_audio else pv_t[j]
                d2 = wk.tile([P, BS], f32, tag="d", name="d2")
                nc.vector.tensor_sub(d2, pre[:, kt, :], tk(t_m, kt))
                nc.vector.tensor_mul(d2, d2, sm)
                nc.vector.tensor_mul(tk(t_m, kt), tk(t_m, kt), bm)
                nc.vector.tensor_add(tk(t_m, kt), tk(t_m, kt), d2)
                if not is_audio:
                    if j == 0:
                        nc.vector.tensor_mul(omt[:, kt, :], cm_t[j], tk(mj, kt))
                    else:
                        tmp2 = wk.tile([P, BS], f32, tag="hm")
                        nc.vector.tensor_mul(tmp2, cm_t[j], tk(mj, kt))
                        nc.vector.tensor_add(omt[:, kt, :], omt[:, kt, :], tmp2)

        for j, mj in enumerate(OTHERS):
            gate_and_mix(j, mj)

        # ---- o2a = others_mean @ o2a_w.T ----
        o2ar = big.tile([P, HT, BS], f32, tag="axr")
        for ho in range(HT):
            ps = ppz.tile([P, BS], f32, tag="z")
            for kt in range(HT):
                wt = wgp.tile([P, P], f32r, tag="ww")
                nc.sync.dma_start(out=wt, in_=O2AT.ap()[ho, kt])
                nc.tensor.matmul(ps, wt, omt[:, kt, :], start=(kt == 0), stop=(kt == HT - 1))
            nc.vector.tensor_copy(o2ar[:, ho, :], ps)
        mix_src["x"] = o2ar

        gate_and_mix(3, AUDIO)

        # ---- LMF ----
        acc = big.tile([P, HT, BS], f32r, tag="acc")
        for r in range(R):
            for ht in range(HT):
                zps = []
                for m in range(M):
                    ft = ftp.tile([P, 9, P], f32r, tag="ft")
                    nc.sync.dma_start(
                        out=ft, in_=FT.ap()[r, ht, m].rearrange("kt p c -> p kt c")
                    )
                    zp = ppz.tile([P, BS], f32, tag="z")
                    for kt in range(HT):
                        nc.tensor.matmul(
                            zp, ft[:, kt, :], tk(m, kt), start=(kt == 0), stop=False
                        )
                    nc.tensor.matmul(zp, ft[0:2, 8, :], uvt[m], start=False, stop=True)
                    zps.append(zp)
                s0 = wk.tile([P, BS], f32, tag="s0")
                nc.vector.tensor_copy(s0, zps[0])
                nc.vector.tensor_mul(s0, s0, zps[1])
                nc.vector.tensor_mul(s0, s0, zps[2])
                if r == 0:
                    nc.vector.tensor_mul(acc[:, ht, :], s0, zps[3])
                else:
                    nc.vector.tensor_mul(s0, s0, zps[3])
                    nc.vector.tensor_add(acc[:, ht, :], acc[:, ht, :], s0)

        # ---- output MLP ----
        stat3 = (pps.tile([1, BS], f32, tag="statA", name="stat3A"),
            pps.tile([1, BS], f32, tag="statB", name="stat3B"))
        pairs3 = []
        for kt in range(HT):
            nc.vector.tensor_scalar_add(
                acc[:, kt, :], acc[:, kt, :], lnv[:, 72 + kt : 72 + kt + 1]
            )
            sq = sqp.tile([P, BS], f32r, tag="sq")
            nc.vector.tensor_mul(sq, acc[:, kt, :], acc[:, kt, :])
            pairs3.append((acc[:, kt, :], sq))
        colsum_stats(stat3, pairs3)
        negmu3, rinv3 = ln_rows(stat3, H, "l1")
        mb3 = bcast(negmu3.bitcast(f32), "mb")
        rb3 = bcast(rinv3, "rb2")
        for kt in range(HT):
            nc.vector.tensor_add(acc[:, kt, :], acc[:, kt, :], mb3)
            nc.vector.tensor_mul(acc[:, kt, :], acc[:, kt, :], rb3)
            nc.vector.tensor_scalar(
                acc[:, kt, :], acc[:, kt, :],
                lnv[:, 32 + kt : 32 + kt + 1], lnv[:, 40 + kt : 40 + kt + 1],
                op0=OP.mult, op1=OP.add,
            )

        # h2 = gelu(h1 @ out_w.T + out_b); LN2; write out
        h2 = big.tile([P, HT, BS], f32r, tag="abs")
        stat4 = (pps.tile([1, BS], f32, tag="statA", name="stat4A"),
            pps.tile([1, BS], f32, tag="statB", name="stat4B"))
        pairs4 = []
        for ho in range(HT):
            ps = ppz.tile([P, BS], f32, tag="z")
            for kt in range(HT):
                wt = wgp.tile([P, P], f32r, tag="ww")
                nc.sync.dma_start(out=wt, in_=OUTWT.ap()[ho, kt])
                nc.tensor.matmul(ps, wt, acc[:, kt, :], start=(kt == 0), stop=(kt == HT - 1))
            nc.scalar.activation(
                h2[:, ho, :], ps, AF.Gelu, bias=lnv[:, 64 + ho : 64 + ho + 1], scale=1.0
            )
            sq = sqp.tile([P, BS], f32r, tag="sq")
            nc.vector.tensor_mul(sq, h2[:, ho, :], h2[:, ho, :])
            pairs4.append((h2[:, ho, :], sq))
        colsum_stats(stat4, pairs4)
        negmu4, rinv4 = ln_rows(stat4, H, "l2")
        mb4 = bcast(negmu4.bitcast(f32), "mb")
        rb4 = bcast(rinv4, "rb2")
        for kt in range(HT):
            fin = wk.tile([P, BS], f32, tag="fin")
            nc.vector.tensor_add(fin, h2[:, kt, :], mb4)
            nc.vector.tensor_mul(fin, fin, rb4)
            nc.vector.tensor_scalar(
                fin, fin, lnv[:, 48 + kt : 48 + kt + 1], lnv[:, 56 + kt : 56 + kt + 1],
                op0=OP.mult, op1=OP.add,
            )
            nc.sync.dma_start(out=OUT.ap()[kt * P : (kt + 1) * P, :], in_=fin)

    nc.compile()
    return nc


def _host_prep(inputs):
    tokens = np.asarray(inputs["tokens"], np.float32)
    token_mask = np.asarray(inputs["token_mask"])
    mask_f = token_mask.astype(np.float32)

    mo = mask_f[:, list(OTHERS)]                      # [B,3]
    ma = mask_f[:, AUDIO]                             # [B]
    pv = mo * ma[:, None]                             # [B,3]
    winv = (1.0 / np.clip(mo.sum(1), 1.0, None)).astype(np.float32)
    aum = ma * (mo.max(1) > 0)                        # [B]

    go_w1 = np.asarray(inputs["go_w1"], np.float32)
    ga_w1 = np.asarray(inputs["ga_w1"], np.float32)

    def gate_prep(w1, b1, lnw, lnb):
        W1w = w1 * lnw[None, :]                       # [MID, 3H]
        c1 = np.ascontiguousarray(W1w.sum(1).reshape(1, MID))
        cb = w1 @ lnb + b1                            # [MID]
        Wblocks = np.ascontiguousarray(W1w.T).reshape(3 * HT, P, MID)
        return Wblocks, c1, cb

    WGOv, c1go, cbgo = gate_prep(
        go_w1, np.asarray(inputs["go_b1"], np.float32),
        np.asarray(inputs["ln_go_w"], np.float32), np.asarray(inputs["ln_go_b"], np.float32),
    )
    WGAv, c1ga, cbga = gate_prep(
        ga_w1, np.asarray(inputs["ga_b1"], np.float32),
        np.asarray(inputs["ln_ga_w"], np.float32), np.asarray(inputs["ln_ga_b"], np.float32),
    )
    CBv = np.ascontiguousarray(
        np.concatenate([cbgo.reshape(MT, P).T, cbga.reshape(MT, P).T], axis=1)
    ).astype(np.float32)                              # [P, 8]
    W2v = np.stack(
        [np.asarray(inputs["go_w2"], np.float32).reshape(MID),
         np.asarray(inputs["ga_w2"], np.float32).reshape(MID)], axis=1
    )                                                 # [MID, 2]
    W2v = np.ascontiguousarray(W2v.reshape(MT, P, 2).transpose(1, 0, 2))
    SCv = np.zeros((1, 8), np.float32)
    SCv[0, 0] = np.asarray(inputs["go_b2"], np.float32).reshape(-1)[0]
    SCv[0, 1] = np.asarray(inputs["ga_b2"], np.float32).reshape(-1)[0]
    SCv[0, 2] = EPS

    def tile_blocks(w):
        wt = np.ascontiguousarray(np.asarray(w, np.float32).T)    # [H_in, H_out]
        return np.ascontiguousarray(wt.reshape(HT, P, HT, P).transpose(2, 0, 1, 3))

    A2OTv = tile_blocks(inputs["a2o_w"])
    O2ATv = tile_blocks(inputs["o2a_w"])
    OUTWTv = tile_blocks(inputs["out_w"])

    def cols(name):
        return np.asarray(inputs[name], np.float32).reshape(HT, P).T

    LNVv = np.zeros((P, 80), np.float32)
    for i, name in enumerate(
        ["ln_o_w", "ln_o_b", "ln_a_w", "ln_a_b", "out_ln1_w", "out_ln1_b",
         "out_ln2_w", "out_ln2_b", "out_b", "lmf_bias"]
    ):
        LNVv[:, 8 * i : 8 * (i + 1)] = cols(name)

    factors = np.asarray(inputs["factors"], np.float32)
    rank_w = np.asarray(inputs["rank_w"], np.float32)
    Ff = factors.copy()
    Ff[AUDIO] = Ff[AUDIO] * rank_w[:, None, None]
    FTv = np.zeros((R, HT, M, 9, P, P), np.float32)
    main = Ff[:, :, 1:, :].reshape(M, R, HT, P, HT, P)   # [m, r, kt, pk, ht, ph]
    FTv[:, :, :, :8, :, :] = main.transpose(1, 4, 0, 2, 3, 5)
    bias = Ff[:, :, 0, :].reshape(M, R, HT, P)           # [m, r, ht, ph]
    FTv[:, :, :, 8, 0, :] = bias.transpose(1, 2, 0, 3)
    ones_row = np.ones((R, HT, M, P), np.float32)
    ones_row[:, :, AUDIO, :] = rank_w[:, None, None]
    FTv[:, :, :, 8, 1, :] = ones_row

    shared = dict(
        WGO=WGOv, WGA=WGAv, WGOe=c1go, WGAe=c1ga, W2=W2v, CB=CBv, SC=SCv,
        A2OT=A2OTv, O2AT=O2ATv, OUTWT=OUTWTv, LNV=LNVv, FT=FTv,
        KON=np.ones((P, 1), np.float32),
    )

    in_maps = []
    for c in range(NCORES):
        sl = slice(c * BS, (c + 1) * BS)
        tokTv = np.ascontiguousarray(tokens[sl].transpose(1, 2, 0))  # [M, H, BS]
        u8v = np.zeros((8, BS), np.uint8)
        u8v[0:3] = pv[sl].T > 0
        u8v[3:6] = mo[sl].T > 0
        u8v[6] = aum[sl] > 0
        u8v[7] = ma[sl] > 0
        cmv = np.ascontiguousarray((mo[sl] * winv[sl, None]).T.astype(np.float32))
        uvv = np.zeros((M, 2, BS), np.float32)
        uvv[:, 0, :] = mask_f[sl].T
        uvv[:, 1, :] = 1.0 - mask_f[sl].T
        in_maps.append(dict(tokT=tokTv, u8rows=u8v, cmrows=cmv, uv=uvv, **shared))
    return in_maps


def kernel(**inputs):
    global _cached_nc, LAST_RESULTS
    if _cached_nc is None:
        _cached_nc = _build()
    in_maps = _host_prep(inputs)
    res = run_bass_kernel_spmd(
        _cached_nc, in_maps, core_ids=list(range(NCORES)), trace=TRACE
    )
    LAST_RESULTS = res
    out = np.stack([res.results[c]["outT"].T for c in range(NCORES)], axis=0)
    return np.ascontiguousarray(out.reshape(B, H)).astype(np.float32)



# revision 11
# speedup vs baseline: 2.0031x; 2.0031x over previous
"""TRN2 Bass kernel for nn_BlendEmoBackbone: gated audio mixer + low-rank
multiplicative fusion, data-parallel over batch on 8 NeuronCores.

Strategy (v2, bf16):
- Pure data parallel: each core handles B/8 = 512 batch rows; gate MLP
  weights and LMF factor tensors replicated (bf16 halves HBM traffic).
- All activations in transposed [feature, batch] layout; every matmul
  contracts over the partition dim. bf16 operands stream 1 cycle/row on
  the PE (fp32/f32r streams at ~2 cycles/row on real TRN2).
- LayerNorm stats via PE ones-matmul column sums; -mu folded into gate
  matmuls as an extra K=1 row.
- LMF where(mask, z, 1) + x_aug ones-column folded into a K=2 tail tile
  in the same psum chain; rank_w folded into the audio factor slices.
- Factors stored partition-major [R,HT,M,P,9,P] so each (r,ht) loads
  with ONE contiguous-per-partition DMA (2.3KB lines).
- WGO gate weights resident in SBUF (single DMA, reused by 3 gates);
  the audio-source half of the other-gate matmul computed once (S).
- Row->tile broadcasts on gpsimd (partition_broadcast); abs/gelu/
  sigmoid/psum-copies on the scalar engine; products/blends on DVE in
  bf16 where precision allows.
"""

import numpy as np
import ml_dtypes
from contextlib import ExitStack

import concourse.bass as bass
from concourse import bacc
import concourse.tile as tile
from concourse import mybir
from concourse.bass_utils import run_bass_kernel_spmd

B, M, H, R = 4096, 4, 1024, 10
NCORES = 8
BS = B // NCORES          # 512 batch rows per core
MID = 512
P = 128
HT = H // P               # 8 h-tiles
MT = MID // P             # 4 mid-tiles
D3 = 3 * H
OTHERS = (0, 2, 3)
AUDIO = 1
EPS = 1e-5

f32 = mybir.dt.float32
f32r = mybir.dt.float32r
bf16 = mybir.dt.bfloat16
u8 = mybir.dt.uint8
AF = mybir.ActivationFunctionType
OP = mybir.AluOpType
bfnp = ml_dtypes.bfloat16

TRACE = False
LAST_RESULTS = None

_cached_nc = None


def _build():
    nc = bacc.Bacc("TRN2", target_bir_lowering=False, debug=False)

    # ---- DRAM parameters (per core) ----
    tokT = nc.declare_dram_parameter("tokT", [M, H, BS], bf16, isOutput=False)
    # u8 rows: 0-2 pv_j, 3 am(aum)
    u8rows = nc.declare_dram_parameter("u8rows", [4, BS], u8, isOutput=False)
    # bf16 rows: 0-2 mo_j, 3 ma, 4-6 cm_j
    f16rows = nc.declare_dram_parameter("f16rows", [7, BS], bf16, isOutput=False)
    uv = nc.declare_dram_parameter("uv", [M, 2, BS], bf16, isOutput=False)
    WGO = nc.declare_dram_parameter("WGO", [3 * HT, P, MID], bf16, isOutput=False)
    WGA = nc.declare_dram_parameter("WGA", [3 * HT, P, MID], bf16, isOutput=False)
    WGOe = nc.declare_dram_parameter("WGOe", [1, MID], bf16, isOutput=False)
    WGAe = nc.declare_dram_parameter("WGAe", [1, MID], bf16, isOutput=False)
    W2 = nc.declare_dram_parameter("W2", [P, MT, 2], bf16, isOutput=False)
    CB = nc.declare_dram_parameter("CB", [P, 8], f32, isOutput=False)
    SC = nc.declare_dram_parameter("SC", [1, 8], f32, isOutput=False)
    # [ht_out, kt, P, P] tiled weight blocks (lhsT layout)
    A2OT = nc.declare_dram_parameter("A2OT", [HT, HT, P, P], bf16, isOutput=False)
    O2AT = nc.declare_dram_parameter("O2AT", [HT, HT, P, P], bf16, isOutput=False)
    OUTWT = nc.declare_dram_parameter("OUTWT", [HT, HT, P, P], bf16, isOutput=False)
    # cols: ln_o_w 0:8, ln_o_b 8:16, ln_a_w 16:24, ln_a_b 24:32,
    #       ln1w 32:40, ln1b 40:48, ln2w 48:56, ln2b 56:64, outb 64:72, lmfb 72:80
    LNV = nc.declare_dram_parameter("LNV", [P, 80], f32, isOutput=False)
    # partition-major factor blocks; [.., p, 0:8, :] = main k-tiles,
    # [.., 0:2, 8, :] = [bias_row; ones_or_rankw_row]
    FT = nc.declare_dram_parameter("FT", [R, HT, M, P, 9, P], bf16, isOutput=False)
    OUT = nc.declare_dram_parameter("outT", [H, BS], f32, isOutput=True)

    with tile.TileContext(nc) as tc, ExitStack() as ctx:
        kp = ctx.enter_context(tc.tile_pool(name="konst", bufs=1))
        tokp = ctx.enter_context(tc.tile_pool(name="tokp", bufs=1))
        big = ctx.enter_context(tc.tile_pool(name="big", bufs=1))
        wk = ctx.enter_context(tc.tile_pool(name="wk", bufs=2))
        bcp = ctx.enter_context(tc.tile_pool(name="bcp", bufs=1))
        sqp = ctx.enter_context(tc.tile_pool(name="sqp", bufs=3))
        wgp = ctx.enter_context(tc.tile_pool(name="wgp", bufs=2))
        ftp = ctx.enter_context(tc.tile_pool(name="ftp", bufs=2))
        rowp = ctx.enter_context(tc.tile_pool(name="rowp", bufs=1))
        ppz = ctx.enter_context(tc.tile_pool(name="ppz", bufs=4, space="PSUM"))
        pps = ctx.enter_context(tc.tile_pool(name="pps", bufs=2, space="PSUM"))

        # ---- constants / small loads ----
        ones_k = kp.tile([P, 1], bf16)
        nc.vector.memset(ones_k, 1.0)
        ones_kf32 = kp.tile([P, 1], f32)
        nc.vector.memset(ones_kf32, 1.0)
        ones_kf = ones_kf32.bitcast(f32r)

        def bc_row_dma(dst, src_ap):
            nc.sync.dma_start(
                out=dst,
                in_=bass.AP(
                    tensor=src_ap.tensor, offset=src_ap.offset, ap=[[0, P], [1, BS]]
                ),
            )

        u8t = []
        for i in range(4):
            t = kp.tile([P, BS], u8, tag=f"u8_{i}")
            bc_row_dma(t, u8rows.ap()[i : i + 1, :])
            u8t.append(t)
        pv_t, am_t = u8t[0:3], u8t[3]
        f16t = []
        for i in range(7):
            t = kp.tile([P, BS], bf16, tag=f"f16_{i}")
            bc_row_dma(t, f16rows.ap()[i : i + 1, :])
            f16t.append(t)
        mo_t, ma_t, cm_t = f16t[0:3], f16t[3], f16t[4:7]
        uvt = []
        for m in range(M):
            t = kp.tile([2, BS], bf16, tag=f"uv_{m}")
            nc.sync.dma_start(out=t, in_=uv.ap()[m])
            uvt.append(t)
        cbt = kp.tile([P, 8], f32)
        nc.sync.dma_start(out=cbt, in_=CB.ap())
        sct = kp.tile([1, 8], f32)
        nc.sync.dma_start(out=sct, in_=SC.ap())
        lnv = kp.tile([P, 80], f32)
        nc.sync.dma_start(out=lnv, in_=LNV.ap())
        w2t = kp.tile([P, MT, 2], bf16)
        nc.sync.dma_start(out=w2t, in_=W2.ap())
        wgoe = kp.tile([1, MID], bf16)
        nc.sync.dma_start(out=wgoe, in_=WGOe.ap())
        wgae = kp.tile([1, MID], bf16)
        nc.sync.dma_start(out=wgae, in_=WGAe.ap())

        # resident other-gate weights (reused by 3 gates): one DMA
        wgo_sb = kp.tile([P, 3 * HT, MID], bf16)
        nc.sync.dma_start(out=wgo_sb, in_=WGO.ap().rearrange("k p c -> p k c"))

        # ---- tokens (transposed, bf16) ----
        tok = tokp.tile([P, M, HT, BS], bf16)
        for m in range(M):
            nc.sync.dma_start(
                out=tok[:, m], in_=tokT.ap()[m].rearrange("(ht p) b -> p ht b", p=P)
            )

        def tk(m, kt):
            return tok[:, m, kt, :]

        def tkw(m):  # whole-token [P, HT, BS] view
            return tok[:, m]

        def flat(t3):
            return t3.rearrange("p a b -> p (a b)")

        def b3(t2):  # [P,BS] -> broadcast [P,HT,BS]
            return t2.unsqueeze(1).broadcast_to([P, HT, BS])

        # ---- helpers ----
        def ln_rows(statA, statB, n, tag, hi_mu=False):
            mdt = f32 if hi_mu else bf16
            mtag = "negmuf" if hi_mu else "negmu"
            negmu = rowp.tile([1, BS], mdt, tag=mtag, name=f"negmu_{tag}")
            nc.scalar.activation(negmu, statA, AF.Copy, bias=0.0, scale=-1.0 / n)
            ex2 = rowp.tile([1, BS], f32, tag="ex2", name=f"ex2_{tag}")
            nc.scalar.activation(ex2, statB, AF.Copy, bias=0.0, scale=1.0 / n)
            msq = rowp.tile([1, BS], f32, tag="msq", name=f"msq_{tag}")
            nc.scalar.activation(msq, negmu, AF.Square)
            nc.vector.tensor_sub(ex2, ex2, msq)                      # var in place
            nc.vector.tensor_scalar_max(ex2, ex2, 0.0)               # bf16 rounding guard
            nc.scalar.activation(msq, ex2, AF.Sqrt, bias=sct[0:1, 2:3], scale=1.0)
            rinv = rowp.tile([1, BS], f32, tag="rinv", name=f"rinv_{tag}")
            nc.vector.reciprocal(rinv, msq)
            rinvb = rowp.tile([1, BS], bf16, tag="rinvb", name=f"rinvb_{tag}")
            nc.scalar.activation(rinvb, rinv, AF.Copy)
            return negmu, rinvb

        def bcast(row, tag, dt=bf16):
            """Broadcast a [1,BS] row to [P,BS] via gpsimd."""
            sb = bcp.tile([P, BS], dt, tag=f"bc_{tag}")
            nc.gpsimd.partition_broadcast(sb, row)
            return sb

        def colsum_sq(statB, tiles3, dt=bf16, name="sq"):
            """statB += per-column sums of squares of all kt slices."""
            chunks = []
            for t3 in tiles3:
                for kt in range(HT):
                    chunks.append(t3[:, kt, :])
            n = len(chunks)
            lhs = ones_k if dt == bf16 else ones_kf
            for i, ch in enumerate(chunks):
                sq = sqp.tile([P, BS], dt, tag=f"sq_{name}")
                nc.vector.tensor_mul(sq, ch, ch)
                nc.tensor.matmul(statB, lhs, sq, start=(i == 0), stop=(i == n - 1))

        def colsum(statA, tiles3, dt=bf16):
            chunks = []
            for t3 in tiles3:
                for kt in range(HT):
                    chunks.append(t3[:, kt, :])
            n = len(chunks)
            lhs = ones_k if dt == bf16 else ones_kf
            for i, ch in enumerate(chunks):
                nc.tensor.matmul(statA, lhs, ch, start=(i == 0), stop=(i == n - 1))

        def linmap(WT, src3, dst3):
            """dst3[ho] = sum_kt WT[ho,kt].T @ src3[kt]; WT streamed from DRAM."""
            for ho in range(HT):
                wt = wgp.tile([P, HT, P], bf16, tag="lin")
                nc.sync.dma_start(out=wt, in_=WT.ap()[ho].rearrange("k p c -> p k c"))
                ps = ppz.tile([P, BS], f32, tag="z")
                for kt in range(HT):
                    nc.tensor.matmul(
                        ps, wt[:, kt, :], src3[:, kt, :],
                        start=(kt == 0), stop=(kt == HT - 1),
                    )
                nc.scalar.activation(dst3[:, ho, :], ps, AF.Copy)

        # ---- a2o = audio @ a2o_w.T, in T layout (bf16) ----
        a2or = big.tile([P, HT, BS], bf16, tag="axr")
        linmap(A2OT, tkw(AUDIO), a2or)

        # audio squares, cached for the 3 other-gates' stat chains
        asq = big.tile([P, HT, BS], bf16, tag="asq")
        nc.vector.tensor_mul(flat(asq), flat(tkw(AUDIO)), flat(tkw(AUDIO)))

        # S_mt = sum_k Wgo_s[k].T @ audio  (shared source half of gate1)
        S = big.tile([P, MT, BS], bf16, tag="Sg")
        for mt in range(MT):
            ps = ppz.tile([P, BS], f32, tag="z")
            for k in range(HT):
                nc.tensor.matmul(
                    ps, wgo_sb[:, HT + k, mt * P : (mt + 1) * P], tk(AUDIO, k),
                    start=(k == 0), stop=(k == HT - 1),
                )
            nc.scalar.activation(S[:, mt, :], ps, AF.Copy)

        omt = big.tile([P, HT, BS], bf16, tag="om")  # others_mean accumulator
        mix_src = {"x": a2or}

        def gate_and_mix(j, mj):
            """j: 0..2 index into OTHERS, or 3 for the audio gate."""
            is_audio = j == 3
            t_m = AUDIO if is_audio else mj
            t3 = tkw(t_m)
            s3 = omt if is_audio else tkw(AUDIO)

            # d = t - s; abs (whole-token merged ops)
            d3 = big.tile([P, HT, BS], bf16, tag="d3")
            nc.vector.tensor_sub(flat(d3), flat(t3), flat(s3))
            abs3 = big.tile([P, HT, BS], bf16, tag="abs")
            nc.scalar.activation(flat(abs3), flat(d3), AF.Abs)

            statA = pps.tile([1, BS], f32, tag="statA", name=f"statA_{j}")
            statB = pps.tile([1, BS], f32, tag="statB", name=f"statB_{j}")
            colsum(statA, [t3, s3, abs3])
            if is_audio:
                colsum_sq(statB, [t3, s3, d3])
            else:
                # t^2 and d^2 computed here; audio^2 reused from asq
                chunks = [t3[:, k, :] for k in range(HT)] + [d3[:, k, :] for k in range(HT)]
                for i, ch in enumerate(chunks):
                    sq = sqp.tile([P, BS], bf16, tag="sq_sq")
                    nc.vector.tensor_mul(sq, ch, ch)
                    nc.tensor.matmul(statB, ones_k, sq, start=(i == 0), stop=False)
                for k in range(HT):
                    nc.tensor.matmul(
                        statB, ones_k, asq[:, k, :], start=False, stop=(k == HT - 1)
                    )
            negmu, rinvb = ln_rows(statA, statB, D3, f"g{j}")

            # gate layer 1
            wge = wgae if is_audio else wgoe
            gps = [ppz.tile([P, BS], f32, tag="z", name=f"gps{mt}") for mt in range(MT)]
            for mt in range(MT):
                cs = slice(mt * P, (mt + 1) * P)
                if is_audio:
                    # stream WGA column-block [3H, P] for this mt, in use order
                    rhs_parts = [t3, s3, abs3]
                    for part in range(3):
                        w = wgp.tile([P, HT, P], bf16, tag="lin", name=f"wga{mt}_{part}")
                        nc.sync.dma_start(
                            out=w,
                            in_=WGA.ap()[
                                part * HT : (part + 1) * HT, :, mt * P : (mt + 1) * P
                            ].rearrange("k p c -> p k c"),
                        )
                        for k in range(HT):
                            nc.tensor.matmul(
                                gps[mt], w[:, k, :], rhs_parts[part][:, k, :],
                                start=(part == 0 and k == 0), stop=False,
                            )
                else:
                    for k in range(HT):
                        nc.tensor.matmul(
                            gps[mt], wgo_sb[:, k, cs], t3[:, k, :],
                            start=(k == 0), stop=False,
                        )
                    for k in range(HT):
                        nc.tensor.matmul(
                            gps[mt], wgo_sb[:, 2 * HT + k, cs], abs3[:, k, :],
                            start=False, stop=False,
                        )
                nc.tensor.matmul(
                    gps[mt], wge[0:1, cs], negmu, start=False, stop=True,
                )
            rb = bcast(rinvb, "rb")
            cb_off = 4 if is_audio else 0
            col = 1 if is_audio else 0
            gp = pps.tile([1, BS], f32, tag="statA", name=f"gp{j}")
            for mt in range(MT):
                hm = wk.tile([P, BS], f32, tag="hm")
                if is_audio:
                    nc.vector.tensor_mul(hm, gps[mt], rb)
                else:
                    nc.vector.tensor_add(hm, gps[mt], S[:, mt, :])
                    nc.vector.tensor_mul(hm, hm, rb)
                hg1 = wk.tile([P, BS], bf16, tag="hg", name=f"hg{mt}")
                nc.scalar.activation(
                    hg1, hm, AF.Gelu,
                    bias=cbt[:, cb_off + mt : cb_off + mt + 1], scale=1.0,
                )
                nc.tensor.matmul(
                    gp, w2t[:, mt, col : col + 1], hg1,
                    start=(mt == 0), stop=(mt == MT - 1),
                )
            g_row = rowp.tile([1, BS], bf16, tag="g_row")
            nc.scalar.activation(
                g_row, gp, AF.Sigmoid, bias=sct[0:1, col : col + 1], scale=1.0,
            )
            gb = bcast(g_row, "gb")

            # pre = t + g * (a2o | o2a); LN over H; blend into tok in place
            src = mix_src["x"]
            pre = big.tile([P, HT, BS], bf16, tag="pre")
            nc.vector.tensor_mul(pre, src, b3(gb))
            nc.vector.tensor_add(flat(pre), flat(pre), flat(t3))
            stat2A = pps.tile([1, BS], f32, tag="statA", name=f"stat2A_{j}")
            stat2B = pps.tile([1, BS], f32, tag="statB", name=f"stat2B_{j}")
            colsum(stat2A, [pre])
            colsum_sq(stat2B, [pre])
            negmu2, rinvb2 = ln_rows(stat2A, stat2B, H, f"u{j}")
            mb = bcast(negmu2, "mb")
            rb2 = bcast(rinvb2, "rb2")
            wcol = 16 if is_audio else 0
            bcol = 24 if is_audio else 8
            sm = am_t if is_audio else pv_t[j]
            bmf = ma_t if is_audio else mo_t[j]
            for kt in range(HT):
                q = wk.tile([P, BS], bf16, tag="qk")
                nc.vector.tensor_add(q, pre[:, kt, :], mb)
                nc.vector.tensor_mul(q, q, rb2)
                nc.vector.tensor_scalar(
                    q, q,
                    lnv[:, wcol + kt : wcol + kt + 1], lnv[:, bcol + kt : bcol + kt + 1],
                    op0=OP.mult, op1=OP.add,
                )
                # blend: tok = bmf * (sm ? q : t)
                nc.vector.copy_predicated(tk(t_m, kt), sm, q)
                nc.vector.tensor_mul(tk(t_m, kt), tk(t_m, kt), bmf)
                if not is_audio:
                    if j == 0:
                        nc.vector.tensor_mul(omt[:, kt, :], tk(t_m, kt), cm_t[j])
                    else:
                        tmp2 = wk.tile([P, BS], bf16, tag="tmp2")
                        nc.vector.tensor_mul(tmp2, tk(t_m, kt), cm_t[j])
                        nc.vector.tensor_add(omt[:, kt, :], omt[:, kt, :], tmp2)

        for j, mj in enumerate(OTHERS):
            gate_and_mix(j, mj)

        # ---- o2a = others_mean @ o2a_w.T ----
        o2ar = big.tile([P, HT, BS], bf16, tag="axr", name="o2ar")
        linmap(O2AT, omt, o2ar)
        mix_src["x"] = o2ar

        gate_and_mix(3, AUDIO)

        # ---- LMF ----
        acc = big.tile([P, HT, BS], f32r, tag="acc")
        for r in range(R):
            for ht in range(HT):
                ft = ftp.tile([P, M, 9, P], bf16, tag="ft")
                nc.sync.dma_start(
                    out=ft, in_=FT.ap()[r, ht].rearrange("m p k c -> p m k c")
                )
                zps = []
                for m in range(M):
                    zp = ppz.tile([P, BS], f32, tag="z")
                    for kt in range(HT):
                        nc.tensor.matmul(
                            zp, ft[:, m, kt, :], tk(m, kt), start=(kt == 0), stop=False
                        )
                    nc.tensor.matmul(zp, ft[0:2, m, 8, :], uvt[m], start=False, stop=True)
                    zps.append(zp)
                s0 = wk.tile([P, BS], f32, tag="s0")
                nc.scalar.activation(s0, zps[0], AF.Copy)
                nc.vector.tensor_mul(s0, s0, zps[1])
                nc.vector.tensor_mul(s0, s0, zps[2])
                if r == 0:
                    nc.vector.tensor_mul(acc[:, ht, :], s0, zps[3])
                else:
                    nc.vector.tensor_mul(s0, s0, zps[3])
                    nc.vector.tensor_add(acc[:, ht, :], acc[:, ht, :], s0)

        # ---- output MLP ----
        for kt in range(HT):
            nc.vector.tensor_scalar_add(
                acc[:, kt, :], acc[:, kt, :], lnv[:, 72 + kt : 72 + kt + 1]
            )
        stat3A = pps.tile([1, BS], f32, tag="statA", name="stat3A")
        stat3B = pps.tile([1, BS], f32, tag="statB", name="stat3B")
        colsum(stat3A, [acc], dt=f32r)
        colsum_sq(stat3B, [acc], dt=f32r, name="f")
        negmu3, rinvb3 = ln_rows(stat3A, stat3B, H, "l1", hi_mu=True)
        mb3 = bcast(negmu3, "mbf", dt=f32)
        rb3 = bcast(rinvb3, "rb2")
        h1 = big.tile([P, HT, BS], bf16, tag="pre", name="h1")
        nc.vector.tensor_add(h1, acc, b3(mb3))
        nc.vector.tensor_mul(h1, h1, b3(rb3))
        for kt in range(HT):
            nc.vector.tensor_scalar(
                h1[:, kt, :], h1[:, kt, :],
                lnv[:, 32 + kt : 32 + kt + 1], lnv[:, 40 + kt : 40 + kt + 1],
                op0=OP.mult, op1=OP.add,
            )

        # h2 = gelu(h1 @ out_w.T + out_b); LN2; write out
        h2 = big.tile([P, HT, BS], bf16, tag="abs", name="h2")
        for ho in range(HT):
            wt = wgp.tile([P, HT, P], bf16, tag="lin", name=f"ow{ho}")
            nc.sync.dma_start(out=wt, in_=OUTWT.ap()[ho].rearrange("k p c -> p k c"))
            ps = ppz.tile([P, BS], f32, tag="z")
            for kt in range(HT):
                nc.tensor.matmul(
                    ps, wt[:, kt, :], h1[:, kt, :],
                    start=(kt == 0), stop=(kt == HT - 1),
                )
            nc.scalar.activation(
                h2[:, ho, :], ps, AF.Gelu, bias=lnv[:, 64 + ho : 64 + ho + 1], scale=1.0
            )
        stat4A = pps.tile([1, BS], f32, tag="statA", name="stat4A")
        stat4B = pps.tile([1, BS], f32, tag="statB", name="stat4B")
        colsum(stat4A, [h2])
        colsum_sq(stat4B, [h2])
        negmu4, rinvb4 = ln_rows(stat4A, stat4B, H, "l2", hi_mu=True)
        mb4 = bcast(negmu4, "mbf", dt=f32)
        rb4 = bcast(rinvb4, "rb2")
        for kt in range(HT):
            fin = wk.tile([P, BS], f32, tag="fin")
            nc.vector.tensor_add(fin, h2[:, kt, :], mb4)
            nc.vector.tensor_mul(fin, fin, rb4)
            nc.vector.tensor_scalar(
                fin, fin, lnv[:, 48 + kt : 48 + kt + 1], lnv[:, 56 + kt : 56 + kt + 1],
                op0=OP.mult, op1=OP.add,
            )
            nc.sync.dma_start(out=OUT.ap()[kt * P : (kt + 1) * P, :], in_=fin)

    nc.compile()
    return nc


def _host_prep(inputs):
    tokens = np.asarray(inputs["tokens"], np.float32)
    token_mask = np.asarray(inputs["token_mask"])
    mask_f = token_mask.astype(np.float32)

    mo = mask_f[:, list(OTHERS)]                      # [B,3]
    ma = mask_f[:, AUDIO]                             # [B]
    pv = mo * ma[:, None]                             # [B,3]
    winv = (1.0 / np.clip(mo.sum(1), 1.0, None)).astype(np.float32)
    aum = ma * (mo.max(1) > 0)                        # [B]

    go_w1 = np.asarray(inputs["go_w1"], np.float32)
    ga_w1 = np.asarray(inputs["ga_w1"], np.float32)

    def gate_prep(w1, b1, lnw, lnb):
        W1w = w1 * lnw[None, :]                       # [MID, 3H]
        c1 = np.ascontiguousarray(W1w.sum(1).reshape(1, MID))
        cb = w1 @ lnb + b1                            # [MID]
        Wblocks = np.ascontiguousarray(W1w.T).reshape(3 * HT, P, MID)
        return Wblocks, c1, cb

    WGOv, c1go, cbgo = gate_prep(
        go_w1, np.asarray(inputs["go_b1"], np.float32),
        np.asarray(inputs["ln_go_w"], np.float32), np.asarray(inputs["ln_go_b"], np.float32),
    )
    WGAv, c1ga, cbga = gate_prep(
        ga_w1, np.asarray(inputs["ga_b1"], np.float32),
        np.asarray(inputs["ln_ga_w"], np.float32), np.asarray(inputs["ln_ga_b"], np.float32),
    )
    CBv = np.ascontiguousarray(
        np.concatenate([cbgo.reshape(MT, P).T, cbga.reshape(MT, P).T], axis=1)
    ).astype(np.float32)                              # [P, 8]
    W2v = np.stack(
        [np.asarray(inputs["go_w2"], np.float32).reshape(MID),
         np.asarray(inputs["ga_w2"], np.float32).reshape(MID)], axis=1
    )                                                 # [MID, 2]
    W2v = np.ascontiguousarray(W2v.reshape(MT, P, 2).transpose(1, 0, 2))
    SCv = np.zeros((1, 8), np.float32)
    SCv[0, 0] = np.asarray(inputs["go_b2"], np.float32).reshape(-1)[0]
    SCv[0, 1] = np.asarray(inputs["ga_b2"], np.float32).reshape(-1)[0]
    SCv[0, 2] = EPS

    def tile_blocks(w):
        wt = np.ascontiguousarray(np.asarray(w, np.float32).T)    # [H_in, H_out]
        return np.ascontiguousarray(
            wt.reshape(HT, P, HT, P).transpose(2, 0, 1, 3)
        ).astype(bfnp)

    A2OTv = tile_blocks(inputs["a2o_w"])
    O2ATv = tile_blocks(inputs["o2a_w"])
    OUTWTv = tile_blocks(inputs["out_w"])

    def cols(name):
        return np.asarray(inputs[name], np.float32).reshape(HT, P).T

    LNVv = np.zeros((P, 80), np.float32)
    for i, name in enumerate(
        ["ln_o_w", "ln_o_b", "ln_a_w", "ln_a_b", "out_ln1_w", "out_ln1_b",
         "out_ln2_w", "out_ln2_b", "out_b", "lmf_bias"]
    ):
        LNVv[:, 8 * i : 8 * (i + 1)] = cols(name)

    factors = np.asarray(inputs["factors"], np.float32)
    rank_w = np.asarray(inputs["rank_w"], np.float32)
    Ff = factors.copy()
    Ff[AUDIO] = Ff[AUDIO] * rank_w[:, None, None]
    # partition-major layout [R, HT, M, P, 9, P]
    FTv = np.zeros((R, HT, M, P, 9, P), np.float32)
    main = Ff[:, :, 1:, :].reshape(M, R, HT, P, HT, P)   # [m, r, kt, pk, ht, ph]
    FTv[:, :, :, :, :8, :] = main.transpose(1, 4, 0, 3, 2, 5)
    bias = Ff[:, :, 0, :].reshape(M, R, HT, P)           # [m, r, ht, ph]
    FTv[:, :, :, 0, 8, :] = bias.transpose(1, 2, 0, 3)
    ones_row = np.ones((R, HT, M, P), np.float32)
    ones_row[:, :, AUDIO, :] = rank_w[:, None, None]
    FTv[:, :, :, 1, 8, :] = ones_row
    FTv = FTv.astype(bfnp)

    shared = dict(
        WGO=WGOv.astype(bfnp), WGA=WGAv.astype(bfnp),
        WGOe=c1go.astype(bfnp), WGAe=c1ga.astype(bfnp),
        W2=W2v.astype(bfnp), CB=CBv, SC=SCv,
        A2OT=A2OTv, O2AT=O2ATv, OUTWT=OUTWTv, LNV=LNVv, FT=FTv,
    )

    in_maps = []
    for c in range(NCORES):
        sl = slice(c * BS, (c + 1) * BS)
        tokTv = np.ascontiguousarray(tokens[sl].transpose(1, 2, 0)).astype(bfnp)
        u8v = np.zeros((4, BS), np.uint8)
        u8v[0:3] = pv[sl].T > 0
        u8v[3] = aum[sl] > 0
        f16v = np.zeros((7, BS), np.float32)
        f16v[0:3] = mo[sl].T
        f16v[3] = ma[sl]
        f16v[4:7] = (mo[sl] * winv[sl, None]).T
        uvv = np.zeros((M, 2, BS), np.float32)
        uvv[:, 0, :] = mask_f[sl].T
        uvv[:, 1, :] = 1.0 - mask_f[sl].T
        in_maps.append(dict(
            tokT=tokTv, u8rows=u8v, f16rows=f16v.astype(bfnp),
            uv=uvv.astype(bfnp), **shared,
        ))
    return in_maps


def kernel(**inputs):
    global _cached_nc, LAST_RESULTS
    if _cached_nc is None:
        _cached_nc = _build()
    in_maps = _host_prep(inputs)
    res = run_bass_kernel_spmd(
        _cached_nc, in_maps, core_ids=list(range(NCORES)), trace=TRACE
    )
    LAST_RESULTS = res
    out = np.stack([res.results[c]["outT"].T for c in range(NCORES)], axis=0)
    return np.ascontiguousarray(out.reshape(B, H)).astype(np.float32)


# revision 18
# speedup vs baseline: 2.0420x; 1.0194x over previous
"""TRN2 Bass kernel for nn_BlendEmoBackbone: gated audio mixer + low-rank
multiplicative fusion, data-parallel over batch on 8 NeuronCores.

Strategy (v2, bf16):
- Pure data parallel: each core handles B/8 = 512 batch rows; gate MLP
  weights and LMF factor tensors replicated (bf16 halves HBM traffic).
- All activations in transposed [feature, batch] layout; every matmul
  contracts over the partition dim. bf16 operands stream 1 cycle/row on
  the PE (fp32/f32r streams at ~2 cycles/row on real TRN2).
- LayerNorm stats via PE ones-matmul column sums; -mu folded into gate
  matmuls as an extra K=1 row.
- LMF where(mask, z, 1) + x_aug ones-column folded into a K=2 tail tile
  in the same psum chain; rank_w folded into the audio factor slices.
- Factors stored partition-major [R,HT,M,P,9,P] so each (r,ht) loads
  with ONE contiguous-per-partition DMA (2.3KB lines).
- WGO gate weights resident in SBUF (single DMA, reused by 3 gates);
  the audio-source half of the other-gate matmul computed once (S).
- Row->tile broadcasts on gpsimd (partition_broadcast); abs/gelu/
  sigmoid/psum-copies on the scalar engine; products/blends on DVE in
  bf16 where precision allows.
"""

import numpy as np
import ml_dtypes
from contextlib import ExitStack

import concourse.bass as bass
from concourse import bacc
import concourse.tile as tile
from concourse import mybir
from concourse.bass_utils import run_bass_kernel_spmd

B, M, H, R = 4096, 4, 1024, 10
NCORES = 8
BS = B // NCORES          # 512 batch rows per core
MID = 512
P = 128
HT = H // P               # 8 h-tiles
MT = MID // P             # 4 mid-tiles
D3 = 3 * H
OTHERS = (0, 2, 3)
AUDIO = 1
EPS = 1e-5

f32 = mybir.dt.float32
f32r = mybir.dt.float32r
bf16 = mybir.dt.bfloat16
u8 = mybir.dt.uint8
AF = mybir.ActivationFunctionType
OP = mybir.AluOpType
bfnp = ml_dtypes.bfloat16

TRACE = False
LAST_RESULTS = None

_cached_nc = None


def _build():
    nc = bacc.Bacc("TRN2", target_bir_lowering=False, debug=False)

    # ---- DRAM parameters (per core) ----
    tokT = nc.declare_dram_parameter("tokT", [M, H, BS], bf16, isOutput=False)
    # u8 rows: 0-2 pv_j, 3 am(aum)
    u8rows = nc.declare_dram_parameter("u8rows", [4, BS], u8, isOutput=False)
    # bf16 rows: 0-2 mo_j, 3 ma, 4-6 cm_j
    f16rows = nc.declare_dram_parameter("f16rows", [7, BS], bf16, isOutput=False)
    uv = nc.declare_dram_parameter("uv", [M, 2, BS], bf16, isOutput=False)
    WGO = nc.declare_dram_parameter("WGO", [3 * HT, P, MID], bf16, isOutput=False)
    WGA = nc.declare_dram_parameter("WGA", [3 * HT, P, MID], bf16, isOutput=False)
    WGOe = nc.declare_dram_parameter("WGOe", [1, MID], bf16, isOutput=False)
    WGAe = nc.declare_dram_parameter("WGAe", [1, MID], bf16, isOutput=False)
    W2 = nc.declare_dram_parameter("W2", [P, MT, 2], bf16, isOutput=False)
    CB = nc.declare_dram_parameter("CB", [P, 8], f32, isOutput=False)
    SC = nc.declare_dram_parameter("SC", [1, 8], f32, isOutput=False)
    # [ht_out, kt, P, P] tiled weight blocks (lhsT layout)
    A2OT = nc.declare_dram_parameter("A2OT", [HT, HT, P, P], bf16, isOutput=False)
    O2AT = nc.declare_dram_parameter("O2AT", [HT, HT, P, P], bf16, isOutput=False)
    OUTWT = nc.declare_dram_parameter("OUTWT", [HT, HT, P, P], bf16, isOutput=False)
    # cols: ln_o_w 0:8, ln_o_b 8:16, ln_a_w 16:24, ln_a_b 24:32,
    #       ln1w 32:40, ln1b 40:48, ln2w 48:56, ln2b 56:64, outb 64:72, lmfb 72:80
    LNV = nc.declare_dram_parameter("LNV", [P, 80], f32, isOutput=False)
    # partition-major factor blocks; [.., p, 0:8, :] = main k-tiles,
    # [.., 0:2, 8, :] = [bias_row; ones_or_rankw_row]
    FT = nc.declare_dram_parameter("FT", [R, HT, M, P, 9, P], bf16, isOutput=False)
    OUT = nc.declare_dram_parameter("outT", [H, BS], f32, isOutput=True)

    with tile.TileContext(nc) as tc, ExitStack() as ctx:
        kp = ctx.enter_context(tc.tile_pool(name="konst", bufs=1))
        tokp = ctx.enter_context(tc.tile_pool(name="tokp", bufs=1))
        big = ctx.enter_context(tc.tile_pool(name="big", bufs=1))
        wk = ctx.enter_context(tc.tile_pool(name="wk", bufs=2))
        bcp = ctx.enter_context(tc.tile_pool(name="bcp", bufs=1))
        sqp = ctx.enter_context(tc.tile_pool(name="sqp", bufs=3))
        wgp = ctx.enter_context(tc.tile_pool(name="wgp", bufs=2))
        ftp = ctx.enter_context(tc.tile_pool(name="ftp", bufs=2))
        rowp = ctx.enter_context(tc.tile_pool(name="rowp", bufs=1))
        ppz = ctx.enter_context(tc.tile_pool(name="ppz", bufs=6, space="PSUM"))
        pps = ctx.enter_context(tc.tile_pool(name="pps", bufs=1, space="PSUM"))

        # ---- constants / small loads ----
        ones_k = kp.tile([P, 1], bf16)
        nc.vector.memset(ones_k, 1.0)
        ones_kf32 = kp.tile([P, 1], f32)
        nc.vector.memset(ones_kf32, 1.0)
        ones_kf = ones_kf32.bitcast(f32r)

        def bc_row_dma(dst, src_ap):
            nc.sync.dma_start(
                out=dst,
                in_=bass.AP(
                    tensor=src_ap.tensor, offset=src_ap.offset, ap=[[0, P], [1, BS]]
                ),
            )

        u8t = []
        for i in range(4):
            t = kp.tile([P, BS], u8, tag=f"u8_{i}")
            bc_row_dma(t, u8rows.ap()[i : i + 1, :])
            u8t.append(t)
        pv_t, am_t = u8t[0:3], u8t[3]
        f16t = []
        for i in range(7):
            t = kp.tile([P, BS], bf16, tag=f"f16_{i}")
            bc_row_dma(t, f16rows.ap()[i : i + 1, :])
            f16t.append(t)
        mo_t, ma_t, cm_t = f16t[0:3], f16t[3], f16t[4:7]
        uvt = []
        for m in range(M):
            t = kp.tile([2, BS], bf16, tag=f"uv_{m}")
            nc.sync.dma_start(out=t, in_=uv.ap()[m])
            uvt.append(t)
        cbt = kp.tile([P, 8], f32)
        nc.sync.dma_start(out=cbt, in_=CB.ap())
        sct = kp.tile([1, 8], f32)
        nc.sync.dma_start(out=sct, in_=SC.ap())
        lnv = kp.tile([P, 80], f32)
        nc.sync.dma_start(out=lnv, in_=LNV.ap())
        w2t = kp.tile([P, MT, 2], bf16)
        nc.sync.dma_start(out=w2t, in_=W2.ap())
        wgoe = kp.tile([1, MID], bf16)
        nc.sync.dma_start(out=wgoe, in_=WGOe.ap())
        wgae = kp.tile([1, MID], bf16)
        nc.sync.dma_start(out=wgae, in_=WGAe.ap())

        # resident other-gate weights (reused by 3 gates): one DMA
        wgo_sb = kp.tile([P, 3 * HT, MID], bf16)
        nc.sync.dma_start(out=wgo_sb, in_=WGO.ap().rearrange("k p c -> p k c"))

        # ---- tokens (transposed, bf16) ----
        tok = tokp.tile([P, M, HT, BS], bf16)
        for m in range(M):
            nc.sync.dma_start(
                out=tok[:, m], in_=tokT.ap()[m].rearrange("(ht p) b -> p ht b", p=P)
            )

        def tk(m, kt):
            return tok[:, m, kt, :]

        def tkw(m):  # whole-token [P, HT, BS] view
            return tok[:, m]

        def flat(t3):
            return t3.rearrange("p a b -> p (a b)")

        def b3(t2):  # [P,BS] -> broadcast [P,HT,BS]
            return t2.unsqueeze(1).broadcast_to([P, HT, BS])

        # ---- helpers ----
        def ln_rows(statA, statB, n, tag, hi_mu=False, par=0):
            mdt = f32 if hi_mu else bf16
            mtag = "negmuf" if hi_mu else f"negmu{par}"
            negmu = rowp.tile([1, BS], mdt, tag=mtag, name=f"negmu_{tag}")
            nc.scalar.activation(negmu, statA, AF.Copy, bias=0.0, scale=-1.0 / n)
            ex2 = rowp.tile([1, BS], f32, tag="ex2", name=f"ex2_{tag}")
            nc.scalar.activation(ex2, statB, AF.Copy, bias=0.0, scale=1.0 / n)
            msq = rowp.tile([1, BS], f32, tag="msq", name=f"msq_{tag}")
            nc.scalar.activation(msq, negmu, AF.Square)
            nc.vector.tensor_sub(ex2, ex2, msq)                      # var in place
            nc.vector.tensor_scalar_max(ex2, ex2, 0.0)               # bf16 rounding guard
            nc.scalar.activation(msq, ex2, AF.Sqrt, bias=sct[0:1, 2:3], scale=1.0)
            rinv = rowp.tile([1, BS], f32, tag="rinv", name=f"rinv_{tag}")
            nc.vector.reciprocal(rinv, msq)
            rinvb = rowp.tile([1, BS], bf16, tag=f"rinvb{par}", name=f"rinvb_{tag}")
            nc.scalar.activation(rinvb, rinv, AF.Copy)
            return negmu, rinvb

        def bcast(row, tag, dt=bf16):
            """Broadcast a [1,BS] row to [P,BS] via gpsimd."""
            sb = bcp.tile([P, BS], dt, tag=f"bc_{tag}")
            nc.gpsimd.partition_broadcast(sb, row)
            return sb

        def colsum_sq(statB, tiles3, dt=bf16, name="sq"):
            """statB += per-column sums of squares of all kt slices."""
            chunks = []
            for t3 in tiles3:
                for kt in range(HT):
                    chunks.append(t3[:, kt, :])
            n = len(chunks)
            lhs = ones_k if dt == bf16 else ones_kf
            for i, ch in enumerate(chunks):
                sq = sqp.tile([P, BS], dt, tag=f"sq_{name}")
                nc.vector.tensor_mul(sq, ch, ch)
                nc.tensor.matmul(statB, lhs, sq, start=(i == 0), stop=(i == n - 1))

        def colsum(statA, tiles3, dt=bf16):
            chunks = []
            for t3 in tiles3:
                for kt in range(HT):
                    chunks.append(t3[:, kt, :])
            n = len(chunks)
            lhs = ones_k if dt == bf16 else ones_kf
            for i, ch in enumerate(chunks):
                nc.tensor.matmul(statA, lhs, ch, start=(i == 0), stop=(i == n - 1))

        def linmap(WT, src3, dst3):
            """dst3[ho] = sum_kt WT[ho,kt].T @ src3[kt]; WT streamed from DRAM."""
            for ho in range(HT):
                wt = wgp.tile([P, HT, P], bf16, tag="lin")
                nc.sync.dma_start(out=wt, in_=WT.ap()[ho].rearrange("k p c -> p k c"))
                ps = ppz.tile([P, BS], f32, tag="z")
                for kt in range(HT):
                    nc.tensor.matmul(
                        ps, wt[:, kt, :], src3[:, kt, :],
                        start=(kt == 0), stop=(kt == HT - 1),
                    )
                nc.scalar.activation(dst3[:, ho, :], ps, AF.Copy)

        # ---- a2o = audio @ a2o_w.T, in T layout (bf16) ----
        a2or = big.tile([P, HT, BS], bf16, tag="axr")
        linmap(A2OT, tkw(AUDIO), a2or)

        # S_mt = sum_k Wgo_s[k].T @ audio  (shared source half of gate1)
        S = big.tile([P, MT, BS], bf16, tag="Sg")
        for mt in range(MT):
            ps = ppz.tile([P, BS], f32, tag="z")
            for k in range(HT):
                nc.tensor.matmul(
                    ps, wgo_sb[:, HT + k, mt * P : (mt + 1) * P], tk(AUDIO, k),
                    start=(k == 0), stop=(k == HT - 1),
                )
            nc.scalar.activation(S[:, mt, :], ps, AF.Copy)

        omt = big.tile([P, HT, BS], bf16, tag="om")  # others_mean accumulator
        mix_src = {"x": a2or}

        def gate_phase1(j, mj):
            """Stats + LN rows for gate j — independent of other gates'
            phase2, so consecutive gates pipeline on the PE."""
            is_audio = j == 3
            t_m = AUDIO if is_audio else mj
            t3 = tkw(t_m)
            s3 = omt if is_audio else tkw(AUDIO)

            abs3 = big.tile([P, HT, BS], bf16, tag=f"abs{j % 2}", name=f"abs3_{j}")
            statA = pps.tile([1, BS], f32, tag="statA", name=f"statA_{j}")
            statB = pps.tile([1, BS], f32, tag="statB", name=f"statB_{j}")
            # d = t - s per kt; |d| -> abs3; d^2 into statB
            for k in range(HT):
                dk = wk.tile([P, BS], bf16, tag="dk")
                nc.vector.tensor_sub(dk, t3[:, k, :], s3[:, k, :])
                nc.scalar.activation(abs3[:, k, :], dk, AF.Abs)
                sq = sqp.tile([P, BS], bf16, tag="sq_sq")
                nc.vector.tensor_mul(sq, dk, dk)
                nc.tensor.matmul(statB, ones_k, sq, start=(k == 0), stop=False)
            for t_src in (t3, s3):
                for k in range(HT):
                    sq = sqp.tile([P, BS], bf16, tag="sq_sq")
                    nc.vector.tensor_mul(sq, t_src[:, k, :], t_src[:, k, :])
                    nc.tensor.matmul(
                        statB, ones_k, sq,
                        start=False, stop=(t_src is s3 and k == HT - 1),
                    )
            colsum(statA, [t3, s3, abs3])
            negmu, rinvb = ln_rows(statA, statB, D3, f"g{j}", par=j % 2)
            return abs3, negmu, rinvb

        def gate_phase2(j, mj, abs3, negmu, rinvb):
            """Gate matmuls, mix, LN, blend for gate j."""
            is_audio = j == 3
            t_m = AUDIO if is_audio else mj
            t3 = tkw(t_m)
            s3 = omt if is_audio else tkw(AUDIO)

            # gate layer 1
            wge = wgae if is_audio else wgoe
            gps = [ppz.tile([P, BS], f32, tag="z", name=f"gps{mt}") for mt in range(MT)]
            for mt in range(MT):
                cs = slice(mt * P, (mt + 1) * P)
                if is_audio:
                    # stream WGA column-block [3H, P] for this mt, in use order
                    rhs_parts = [t3, s3, abs3]
                    for part in range(3):
                        w = wgp.tile([P, HT, P], bf16, tag="lin", name=f"wga{mt}_{part}")
                        nc.sync.dma_start(
                            out=w,
                            in_=WGA.ap()[
                                part * HT : (part + 1) * HT, :, mt * P : (mt + 1) * P
                            ].rearrange("k p c -> p k c"),
                        )
                        for k in range(HT):
                            nc.tensor.matmul(
                                gps[mt], w[:, k, :], rhs_parts[part][:, k, :],
                                start=(part == 0 and k == 0), stop=False,
                            )
                else:
                    for k in range(HT):
                        nc.tensor.matmul(
                            gps[mt], wgo_sb[:, k, cs], t3[:, k, :],
                            start=(k == 0), stop=False,
                        )
                    for k in range(HT):
                        nc.tensor.matmul(
                            gps[mt], wgo_sb[:, 2 * HT + k, cs], abs3[:, k, :],
                            start=False, stop=False,
                        )
                nc.tensor.matmul(
                    gps[mt], wge[0:1, cs], negmu, start=False, stop=True,
                )
            rb = bcast(rinvb, "rb")
            cb_off = 4 if is_audio else 0
            col = 1 if is_audio else 0
            gp = pps.tile([1, BS], f32, tag="statA", name=f"gp{j}")
            for mt in range(MT):
                hm = wk.tile([P, BS], f32, tag="hm")
                if is_audio:
                    nc.vector.tensor_mul(hm, gps[mt], rb)
                else:
                    nc.vector.tensor_add(hm, gps[mt], S[:, mt, :])
                    nc.vector.tensor_mul(hm, hm, rb)
                hg1 = wk.tile([P, BS], bf16, tag="hg", name=f"hg{mt}")
                nc.scalar.activation(
                    hg1, hm, AF.Gelu,
                    bias=cbt[:, cb_off + mt : cb_off + mt + 1], scale=1.0,
                )
                nc.tensor.matmul(
                    gp, w2t[:, mt, col : col + 1], hg1,
                    start=(mt == 0), stop=(mt == MT - 1),
                )
            g_row = rowp.tile([1, BS], bf16, tag="g_row")
            nc.scalar.activation(
                g_row, gp, AF.Sigmoid, bias=sct[0:1, col : col + 1], scale=1.0,
            )
            gb = bcast(g_row, "gb")

            # pre = t + g * (a2o | o2a); LN over H; blend into tok in place
            src = mix_src["x"]
            pre = big.tile([P, HT, BS], bf16, tag="pre")
            nc.vector.tensor_mul(pre, src, b3(gb))
            nc.vector.tensor_add(flat(pre), flat(pre), flat(t3))
            stat2A = pps.tile([1, BS], f32, tag="statA", name=f"stat2A_{j}")
            stat2B = pps.tile([1, BS], f32, tag="statB", name=f"stat2B_{j}")
            colsum(stat2A, [pre])
            colsum_sq(stat2B, [pre])
            negmu2, rinvb2 = ln_rows(stat2A, stat2B, H, f"u{j}", par=j % 2)
            mb = bcast(negmu2, "mb")
            rb2 = bcast(rinvb2, "rb2")
            wcol = 16 if is_audio else 0
            bcol = 24 if is_audio else 8
            sm = am_t if is_audio else pv_t[j]
            bmf = ma_t if is_audio else mo_t[j]
            for kt in range(HT):
                q = wk.tile([P, BS], bf16, tag="qk")
                nc.vector.tensor_add(q, pre[:, kt, :], mb)
                nc.vector.tensor_mul(q, q, rb2)
                nc.vector.tensor_scalar(
                    q, q,
                    lnv[:, wcol + kt : wcol + kt + 1], lnv[:, bcol + kt : bcol + kt + 1],
                    op0=OP.mult, op1=OP.add,
                )
                # blend: tok = bmf * (sm ? q : t)
                nc.vector.copy_predicated(tk(t_m, kt), sm, q)
                nc.vector.tensor_mul(tk(t_m, kt), tk(t_m, kt), bmf)
                if not is_audio:
                    if j == 0:
                        nc.vector.tensor_mul(omt[:, kt, :], tk(t_m, kt), cm_t[j])
                    else:
                        tmp2 = wk.tile([P, BS], bf16, tag="tmp2")
                        nc.vector.tensor_mul(tmp2, tk(t_m, kt), cm_t[j])
                        nc.vector.tensor_add(omt[:, kt, :], omt[:, kt, :], tmp2)

        # software-pipelined emission: gate j+1's stats overlap gate j's
        # post-matmul chain on the PE.
        p1 = {}
        p1[0] = gate_phase1(0, OTHERS[0])
        p1[1] = gate_phase1(1, OTHERS[1])
        gate_phase2(0, OTHERS[0], *p1[0])
        p1[2] = gate_phase1(2, OTHERS[2])
        gate_phase2(1, OTHERS[1], *p1[1])
        gate_phase2(2, OTHERS[2], *p1[2])

        # ---- o2a = others_mean @ o2a_w.T ----
        o2ar = big.tile([P, HT, BS], bf16, tag="axr", name="o2ar")
        linmap(O2AT, omt, o2ar)
        mix_src["x"] = o2ar

        p1[3] = gate_phase1(3, AUDIO)
        gate_phase2(3, AUDIO, *p1[3])

        # ---- LMF ----
        acc = big.tile([P, HT, BS], f32r, tag="acc")
        for r in range(R):
            for ht in range(HT):
                ft = ftp.tile([P, M, 9, P], bf16, tag="ft")
                nc.sync.dma_start(
                    out=ft, in_=FT.ap()[r, ht].rearrange("m p k c -> p m k c")
                )
                zps = []
                for m in range(M):
                    zp = ppz.tile([P, BS], f32, tag="z")
                    for kt in range(HT):
                        nc.tensor.matmul(
                            zp, ft[:, m, kt, :], tk(m, kt), start=(kt == 0), stop=False
                        )
                    nc.tensor.matmul(zp, ft[0:2, m, 8, :], uvt[m], start=False, stop=True)
                    zps.append(zp)
                s0 = wk.tile([P, BS], f32, tag="s0")
                nc.scalar.activation(s0, zps[0], AF.Copy)
                nc.vector.tensor_mul(s0, s0, zps[1])
                nc.vector.tensor_mul(s0, s0, zps[2])
                if r == 0:
                    nc.vector.tensor_mul(acc[:, ht, :], s0, zps[3])
                else:
                    nc.vector.tensor_mul(s0, s0, zps[3])
                    nc.vector.tensor_add(acc[:, ht, :], acc[:, ht, :], s0)

        # ---- output MLP ----
        for kt in range(HT):
            nc.vector.tensor_scalar_add(
                acc[:, kt, :], acc[:, kt, :], lnv[:, 72 + kt : 72 + kt + 1]
            )
        stat3A = pps.tile([1, BS], f32, tag="statA", name="stat3A")
        stat3B = pps.tile([1, BS], f32, tag="statB", name="stat3B")
        colsum(stat3A, [acc], dt=f32r)
        colsum_sq(stat3B, [acc], dt=f32r, name="f")
        negmu3, rinvb3 = ln_rows(stat3A, stat3B, H, "l1", hi_mu=True)
        mb3 = bcast(negmu3, "mbf", dt=f32)
        rb3 = bcast(rinvb3, "rb2")
        h1 = big.tile([P, HT, BS], bf16, tag="pre", name="h1")
        nc.vector.tensor_add(h1, acc, b3(mb3))
        nc.vector.tensor_mul(h1, h1, b3(rb3))
        for kt in range(HT):
            nc.vector.tensor_scalar(
                h1[:, kt, :], h1[:, kt, :],
                lnv[:, 32 + kt : 32 + kt + 1], lnv[:, 40 + kt : 40 + kt + 1],
                op0=OP.mult, op1=OP.add,
            )

        # h2 = gelu(h1 @ out_w.T + out_b); LN2; write out
        h2 = big.tile([P, HT, BS], bf16, tag="abs0", name="h2")
        for ho in range(HT):
            wt = wgp.tile([P, HT, P], bf16, tag="lin", name=f"ow{ho}")
            nc.sync.dma_start(out=wt, in_=OUTWT.ap()[ho].rearrange("k p c -> p k c"))
            ps = ppz.tile([P, BS], f32, tag="z")
            for kt in range(HT):
                nc.tensor.matmul(
                    ps, wt[:, kt, :], h1[:, kt, :],
                    start=(kt == 0), stop=(kt == HT - 1),
                )
            nc.scalar.activation(
                h2[:, ho, :], ps, AF.Gelu, bias=lnv[:, 64 + ho : 64 + ho + 1], scale=1.0
            )
        stat4A = pps.tile([1, BS], f32, tag="statA", name="stat4A")
        stat4B = pps.tile([1, BS], f32, tag="statB", name="stat4B")
        colsum(stat4A, [h2])
        colsum_sq(stat4B, [h2])
        negmu4, rinvb4 = ln_rows(stat4A, stat4B, H, "l2", hi_mu=True)
        mb4 = bcast(negmu4, "mbf", dt=f32)
        rb4 = bcast(rinvb4, "rb2")
        for kt in range(HT):
            fin = wk.tile([P, BS], f32, tag="fin")
            nc.vector.tensor_add(fin, h2[:, kt, :], mb4)
            nc.vector.tensor_mul(fin, fin, rb4)
            nc.vector.tensor_scalar(
                fin, fin, lnv[:, 48 + kt : 48 + kt + 1], lnv[:, 56 + kt : 56 + kt + 1],
                op0=OP.mult, op1=OP.add,
            )
            nc.sync.dma_start(out=OUT.ap()[kt * P : (kt + 1) * P, :], in_=fin)

    nc.compile()
    return nc


def _host_prep(inputs):
    tokens = np.asarray(inputs["tokens"], np.float32)
    token_mask = np.asarray(inputs["token_mask"])
    mask_f = token_mask.astype(np.float32)

    mo = mask_f[:, list(OTHERS)]                      # [B,3]
    ma = mask_f[:, AUDIO]                             # [B]
    pv = mo * ma[:, None]                             # [B,3]
    winv = (1.0 / np.clip(mo.sum(1), 1.0, None)).astype(np.float32)
    aum = ma * (mo.max(1) > 0)                        # [B]

    go_w1 = np.asarray(inputs["go_w1"], np.float32)
    ga_w1 = np.asarray(inputs["ga_w1"], np.float32)

    def gate_prep(w1, b1, lnw, lnb):
        W1w = w1 * lnw[None, :]                       # [MID, 3H]
        c1 = np.ascontiguousarray(W1w.sum(1).reshape(1, MID))
        cb = w1 @ lnb + b1                            # [MID]
        Wblocks = np.ascontiguousarray(W1w.T).reshape(3 * HT, P, MID)
        return Wblocks, c1, cb

    WGOv, c1go, cbgo = gate_prep(
        go_w1, np.asarray(inputs["go_b1"], np.float32),
        np.asarray(inputs["ln_go_w"], np.float32), np.asarray(inputs["ln_go_b"], np.float32),
    )
    WGAv, c1ga, cbga = gate_prep(
        ga_w1, np.asarray(inputs["ga_b1"], np.float32),
        np.asarray(inputs["ln_ga_w"], np.float32), np.asarray(inputs["ln_ga_b"], np.float32),
    )
    CBv = np.ascontiguousarray(
        np.concatenate([cbgo.reshape(MT, P).T, cbga.reshape(MT, P).T], axis=1)
    ).astype(np.float32)                              # [P, 8]
    W2v = np.stack(
        [np.asarray(inputs["go_w2"], np.float32).reshape(MID),
         np.asarray(inputs["ga_w2"], np.float32).reshape(MID)], axis=1
    )                                                 # [MID, 2]
    W2v = np.ascontiguousarray(W2v.reshape(MT, P, 2).transpose(1, 0, 2))
    SCv = np.zeros((1, 8), np.float32)
    SCv[0, 0] = np.asarray(inputs["go_b2"], np.float32).reshape(-1)[0]
    SCv[0, 1] = np.asarray(inputs["ga_b2"], np.float32).reshape(-1)[0]
    SCv[0, 2] = EPS

    def tile_blocks(w):
        wt = np.ascontiguousarray(np.asarray(w, np.float32).T)    # [H_in, H_out]
        return np.ascontiguousarray(
            wt.reshape(HT, P, HT, P).transpose(2, 0, 1, 3)
        ).astype(bfnp)

    A2OTv = tile_blocks(inputs["a2o_w"])
    O2ATv = tile_blocks(inputs["o2a_w"])
    OUTWTv = tile_blocks(inputs["out_w"])

    def cols(name):
        return np.asarray(inputs[name], np.float32).reshape(HT, P).T

    LNVv = np.zeros((P, 80), np.float32)
    for i, name in enumerate(
        ["ln_o_w", "ln_o_b", "ln_a_w", "ln_a_b", "out_ln1_w", "out_ln1_b",
         "out_ln2_w", "out_ln2_b", "out_b", "lmf_bias"]
    ):
        LNVv[:, 8 * i : 8 * (i + 1)] = cols(name)

    factors = np.asarray(inputs["factors"], np.float32)
    rank_w = np.asarray(inputs["rank_w"], np.float32)
    Ff = factors.copy()
    Ff[AUDIO] = Ff[AUDIO] * rank_w[:, None, None]
    # partition-major layout [R, HT, M, P, 9, P]
    FTv = np.zeros((R, HT, M, P, 9, P), np.float32)
    main = Ff[:, :, 1:, :].reshape(M, R, HT, P, HT, P)   # [m, r, kt, pk, ht, ph]
    FTv[:, :, :, :, :8, :] = main.transpose(1, 4, 0, 3, 2, 5)
    bias = Ff[:, :, 0, :].reshape(M, R, HT, P)           # [m, r, ht, ph]
    FTv[:, :, :, 0, 8, :] = bias.transpose(1, 2, 0, 3)
    ones_row = np.ones((R, HT, M, P), np.float32)
    ones_row[:, :, AUDIO, :] = rank_w[:, None, None]
    FTv[:, :, :, 1, 8, :] = ones_row
    FTv = FTv.astype(bfnp)

    shared = dict(
        WGO=WGOv.astype(bfnp), WGA=WGAv.astype(bfnp),
        WGOe=c1go.astype(bfnp), WGAe=c1ga.astype(bfnp),
        W2=W2v.astype(bfnp), CB=CBv, SC=SCv,
        A2OT=A2OTv, O2AT=O2ATv, OUTWT=OUTWTv, LNV=LNVv, FT=FTv,
    )

    in_maps = []
    for c in range(NCORES):
        sl = slice(c * BS, (c + 1) * BS)
        tokTv = np.ascontiguousarray(tokens[sl].transpose(1, 2, 0)).astype(bfnp)
        u8v = np.zeros((4, BS), np.uint8)
        u8v[0:3] = pv[sl].T > 0
        u8v[3] = aum[sl] > 0
        f16v = np.zeros((7, BS), np.float32)
        f16v[0:3] = mo[sl].T
        f16v[3] = ma[sl]
        f16v[4:7] = (mo[sl] * winv[sl, None]).T
        uvv = np.zeros((M, 2, BS), np.float32)
        uvv[:, 0, :] = mask_f[sl].T
        uvv[:, 1, :] = 1.0 - mask_f[sl].T
        in_maps.append(dict(
            tokT=tokTv, u8rows=u8v, f16rows=f16v.astype(bfnp),
            uv=uvv.astype(bfnp), **shared,
        ))
    return in_maps


def kernel(**inputs):
    global _cached_nc, LAST_RESULTS
    if _cached_nc is None:
        _cached_nc = _build()
    in_maps = _host_prep(inputs)
    res = run_bass_kernel_spmd(
        _cached_nc, in_maps, core_ids=list(range(NCORES)), trace=TRACE
    )
    LAST_RESULTS = res
    out = np.stack([res.results[c]["outT"].T for c in range(NCORES)], axis=0)
    return np.ascontiguousarray(out.reshape(B, H)).astype(np.float32)


# revision 24
# speedup vs baseline: 2.0760x; 1.0167x over previous
"""TRN2 Bass kernel for nn_BlendEmoBackbone: gated audio mixer + low-rank
multiplicative fusion, data-parallel over batch on 8 NeuronCores.

Strategy (v2, bf16):
- Pure data parallel: each core handles B/8 = 512 batch rows; gate MLP
  weights and LMF factor tensors replicated (bf16 halves HBM traffic).
- All activations in transposed [feature, batch] layout; every matmul
  contracts over the partition dim. bf16 operands stream 1 cycle/row on
  the PE (fp32/f32r streams at ~2 cycles/row on real TRN2).
- LayerNorm stats via PE ones-matmul column sums; -mu folded into gate
  matmuls as an extra K=1 row.
- LMF where(mask, z, 1) + x_aug ones-column folded into a K=2 tail tile
  in the same psum chain; rank_w folded into the audio factor slices.
- Factors stored partition-major [R,HT,M,P,9,P] so each (r,ht) loads
  with ONE contiguous-per-partition DMA (2.3KB lines).
- WGO gate weights resident in SBUF (single DMA, reused by 3 gates);
  the audio-source half of the other-gate matmul computed once (S).
- Row->tile broadcasts on gpsimd (partition_broadcast); abs/gelu/
  sigmoid/psum-copies on the scalar engine; products/blends on DVE in
  bf16 where precision allows.
"""

import numpy as np
import ml_dtypes
from contextlib import ExitStack

import concourse.bass as bass
from concourse import bacc
import concourse.tile as tile
from concourse import mybir
from concourse.bass_utils import run_bass_kernel_spmd

B, M, H, R = 4096, 4, 1024, 10
NCORES = 8
BS = B // NCORES          # 512 batch rows per core
MID = 512
P = 128
HT = H // P               # 8 h-tiles
MT = MID // P             # 4 mid-tiles
D3 = 3 * H
OTHERS = (0, 2, 3)
AUDIO = 1
EPS = 1e-5

f32 = mybir.dt.float32
f32r = mybir.dt.float32r
bf16 = mybir.dt.bfloat16
u8 = mybir.dt.uint8
AF = mybir.ActivationFunctionType
OP = mybir.AluOpType
bfnp = ml_dtypes.bfloat16

TRACE = False
LAST_RESULTS = None

_cached_nc = None


def _build():
    nc = bacc.Bacc("TRN2", target_bir_lowering=False, debug=False)

    # ---- DRAM parameters (per core) ----
    tokT = nc.declare_dram_parameter("tokT", [M, H, BS], bf16, isOutput=False)
    # u8 rows: 0-2 pv_j, 3 am(aum)
    u8rows = nc.declare_dram_parameter("u8rows", [4, BS], u8, isOutput=False)
    # bf16 rows: 0-2 mo_j, 3 ma, 4-6 cm_j
    f16rows = nc.declare_dram_parameter("f16rows", [7, BS], bf16, isOutput=False)
    uv = nc.declare_dram_parameter("uv", [M, 2, BS], bf16, isOutput=False)
    WGO = nc.declare_dram_parameter("WGO", [3 * HT, P, MID], bf16, isOutput=False)
    WGA = nc.declare_dram_parameter("WGA", [3 * HT, P, MID], bf16, isOutput=False)
    WGOe = nc.declare_dram_parameter("WGOe", [1, MID], bf16, isOutput=False)
    WGAe = nc.declare_dram_parameter("WGAe", [1, MID], bf16, isOutput=False)
    W2 = nc.declare_dram_parameter("W2", [P, MT, 2], bf16, isOutput=False)
    CB = nc.declare_dram_parameter("CB", [P, 8], f32, isOutput=False)
    SC = nc.declare_dram_parameter("SC", [1, 8], f32, isOutput=False)
    # [ht_out, kt, P, P] tiled weight blocks (lhsT layout)
    A2OT = nc.declare_dram_parameter("A2OT", [HT, HT, P, P], bf16, isOutput=False)
    O2AT = nc.declare_dram_parameter("O2AT", [HT, HT, P, P], bf16, isOutput=False)
    OUTWT = nc.declare_dram_parameter("OUTWT", [HT, HT, P, P], bf16, isOutput=False)
    # cols: ln_o_w 0:8, ln_o_b 8:16, ln_a_w 16:24, ln_a_b 24:32,
    #       ln1w 32:40, ln1b 40:48, ln2w 48:56, ln2b 56:64, outb 64:72, lmfb 72:80
    LNV = nc.declare_dram_parameter("LNV", [P, 80], f32, isOutput=False)
    # partition-major factor blocks; [.., p, 0:8, :] = main k-tiles,
    # [.., 0:2, 8, :] = [bias_row; ones_or_rankw_row]
    FT = nc.declare_dram_parameter("FT", [R, HT, M, P, 9, P], bf16, isOutput=False)
    OUT = nc.declare_dram_parameter("outT", [H, BS], f32, isOutput=True)

    with tile.TileContext(nc) as tc, ExitStack() as ctx:
        kp = ctx.enter_context(tc.tile_pool(name="konst", bufs=1))
        tokp = ctx.enter_context(tc.tile_pool(name="tokp", bufs=1))
        big = ctx.enter_context(tc.tile_pool(name="big", bufs=1))
        wk = ctx.enter_context(tc.tile_pool(name="wk", bufs=2))
        bcp = ctx.enter_context(tc.tile_pool(name="bcp", bufs=1))
        sqp = ctx.enter_context(tc.tile_pool(name="sqp", bufs=3))
        wgp = ctx.enter_context(tc.tile_pool(name="wgp", bufs=2))
        ftp = ctx.enter_context(tc.tile_pool(name="ftp", bufs=2))
        rowp = ctx.enter_context(tc.tile_pool(name="rowp", bufs=1))
        ppz = ctx.enter_context(tc.tile_pool(name="ppz", bufs=6, space="PSUM"))
        pps = ctx.enter_context(tc.tile_pool(name="pps", bufs=1, space="PSUM"))

        # ---- constants / small loads ----
        ones_k = kp.tile([P, 1], bf16)
        nc.vector.memset(ones_k, 1.0)
        ones_kf32 = kp.tile([P, 1], f32)
        nc.vector.memset(ones_kf32, 1.0)
        ones_kf = ones_kf32.bitcast(f32r)

        def bc_row_dma(dst, src_ap):
            nc.sync.dma_start(
                out=dst,
                in_=bass.AP(
                    tensor=src_ap.tensor, offset=src_ap.offset, ap=[[0, P], [1, BS]]
                ),
            )

        u8t = []
        for i in range(4):
            t = kp.tile([P, BS], u8, tag=f"u8_{i}")
            bc_row_dma(t, u8rows.ap()[i : i + 1, :])
            u8t.append(t)
        pv_t, am_t = u8t[0:3], u8t[3]
        f16t = []
        for i in range(7):
            t = kp.tile([P, BS], bf16, tag=f"f16_{i}")
            bc_row_dma(t, f16rows.ap()[i : i + 1, :])
            f16t.append(t)
        mo_t, ma_t, cm_t = f16t[0:3], f16t[3], f16t[4:7]
        uvt = []
        for m in range(M):
            t = kp.tile([2, BS], bf16, tag=f"uv_{m}")
            nc.sync.dma_start(out=t, in_=uv.ap()[m])
            uvt.append(t)
        cbt = kp.tile([P, 8], f32)
        nc.sync.dma_start(out=cbt, in_=CB.ap())
        sct = kp.tile([1, 8], f32)
        nc.sync.dma_start(out=sct, in_=SC.ap())
        lnv = kp.tile([P, 80], f32)
        nc.sync.dma_start(out=lnv, in_=LNV.ap())
        w2t = kp.tile([P, MT, 2], bf16)
        nc.sync.dma_start(out=w2t, in_=W2.ap())
        wgoe = kp.tile([1, MID], bf16)
        nc.sync.dma_start(out=wgoe, in_=WGOe.ap())
        wgae = kp.tile([1, MID], bf16)
        nc.sync.dma_start(out=wgae, in_=WGAe.ap())



        # ---- tokens (transposed, bf16) ----
        tok = tokp.tile([P, M, HT, BS], bf16)
        for m in range(M):
            nc.sync.dma_start(
                out=tok[:, m], in_=tokT.ap()[m].rearrange("(ht p) b -> p ht b", p=P)
            )

        def tk(m, kt):
            return tok[:, m, kt, :]

        def tkw(m):  # whole-token [P, HT, BS] view
            return tok[:, m]

        def flat(t3):
            return t3.rearrange("p a b -> p (a b)")

        def b3(t2):  # [P,BS] -> broadcast [P,HT,BS]
            return t2.unsqueeze(1).broadcast_to([P, HT, BS])

        # ---- helpers ----
        def ln_rows(statA, statB, n, tag, hi_mu=False, par=0):
            mdt = f32 if hi_mu else bf16
            mtag = "negmuf" if hi_mu else f"negmu{par}"
            negmu = rowp.tile([1, BS], mdt, tag=mtag, name=f"negmu_{tag}")
            nc.scalar.activation(negmu, statA, AF.Copy, bias=0.0, scale=-1.0 / n)
            ex2 = rowp.tile([1, BS], f32, tag="ex2", name=f"ex2_{tag}")
            nc.scalar.activation(ex2, statB, AF.Copy, bias=0.0, scale=1.0 / n)
            msq = rowp.tile([1, BS], f32, tag="msq", name=f"msq_{tag}")
            nc.scalar.activation(msq, negmu, AF.Square)
            nc.vector.tensor_sub(ex2, ex2, msq)                      # var in place
            nc.vector.tensor_scalar_max(ex2, ex2, 0.0)               # bf16 rounding guard
            nc.scalar.activation(msq, ex2, AF.Sqrt, bias=sct[0:1, 2:3], scale=1.0)
            rinv = rowp.tile([1, BS], f32, tag="rinv", name=f"rinv_{tag}")
            nc.vector.reciprocal(rinv, msq)
            rinvb = rowp.tile([1, BS], bf16, tag=f"rinvb{par}", name=f"rinvb_{tag}")
            nc.scalar.activation(rinvb, rinv, AF.Copy)
            return negmu, rinvb

        def bcast(row, tag, dt=bf16):
            """Broadcast a [1,BS] row to [P,BS] via gpsimd."""
            sb = bcp.tile([P, BS], dt, tag=f"bc_{tag}")
            nc.gpsimd.partition_broadcast(sb, row)
            return sb

        def colsum_sq(statB, tiles3, dt=bf16, name="sq"):
            """statB += per-column sums of squares of all kt slices."""
            chunks = []
            for t3 in tiles3:
                for kt in range(HT):
                    chunks.append(t3[:, kt, :])
            n = len(chunks)
            lhs = ones_k if dt == bf16 else ones_kf
            for i, ch in enumerate(chunks):
                sq = sqp.tile([P, BS], dt, tag=f"sq_{name}")
                nc.vector.tensor_mul(sq, ch, ch)
                nc.tensor.matmul(statB, lhs, sq, start=(i == 0), stop=(i == n - 1))

        def colsum(statA, tiles3, dt=bf16):
            chunks = []
            for t3 in tiles3:
                for kt in range(HT):
                    chunks.append(t3[:, kt, :])
            n = len(chunks)
            lhs = ones_k if dt == bf16 else ones_kf
            for i, ch in enumerate(chunks):
                nc.tensor.matmul(statA, lhs, ch, start=(i == 0), stop=(i == n - 1))

        def linmap(WT, src3, dst3):
            """dst3[ho] = sum_kt WT[ho,kt].T @ src3[kt]; WT streamed from DRAM."""
            for ho in range(HT):
                wt = wgp.tile([P, HT, P], bf16, tag="lin")
                nc.sync.dma_start(out=wt, in_=WT.ap()[ho].rearrange("k p c -> p k c"))
                ps = ppz.tile([P, BS], f32, tag="z")
                for kt in range(HT):
                    nc.tensor.matmul(
                        ps, wt[:, kt, :], src3[:, kt, :],
                        start=(kt == 0), stop=(kt == HT - 1),
                    )
                nc.scalar.activation(dst3[:, ho, :], ps, AF.Copy)

        # ---- a2o = audio @ a2o_w.T, in T layout (bf16) ----
        a2or = big.tile([P, HT, BS], bf16, tag="axr")
        linmap(A2OT, tkw(AUDIO), a2or)

        # S_mt = sum_k Wgo_s[k].T @ audio  (shared source half of gate1)
        S = big.tile([P, MT, BS], bf16, tag="Sg")
        for mt in range(MT):
            wS = wgp.tile([P, HT, P], bf16, tag="lin", name=f"wS{mt}")
            nc.sync.dma_start(
                out=wS,
                in_=WGO.ap()[HT : 2 * HT, :, mt * P : (mt + 1) * P].rearrange(
                    "k p c -> p k c"
                ),
            )
            ps = ppz.tile([P, BS], f32, tag="z")
            for k in range(HT):
                nc.tensor.matmul(
                    ps, wS[:, k, :], tk(AUDIO, k),
                    start=(k == 0), stop=(k == HT - 1),
                )
            nc.scalar.activation(S[:, mt, :], ps, AF.Copy)

        omt = big.tile([P, HT, BS], bf16, tag="om")  # others_mean accumulator
        mix_src = {"x": a2or}

        def gate_phase1(j, mj):
            """Stats + LN rows for gate j — independent of other gates'
            phase2, so consecutive gates pipeline on the PE."""
            is_audio = j == 3
            t_m = AUDIO if is_audio else mj
            t3 = tkw(t_m)
            s3 = omt if is_audio else tkw(AUDIO)

            abs3 = big.tile([P, HT, BS], bf16, tag=f"abs{j % 2}", name=f"abs3_{j}")
            statA = pps.tile([1, BS], f32, tag="statA", name=f"statA_{j}")
            statB = pps.tile([1, BS], f32, tag="statB", name=f"statB_{j}")
            # d = t - s per kt; |d| -> abs3; d^2 into statB
            for k in range(HT):
                dk = wk.tile([P, BS], bf16, tag="dk")
                nc.vector.tensor_sub(dk, t3[:, k, :], s3[:, k, :])
                nc.scalar.activation(abs3[:, k, :], dk, AF.Abs)
                sq = sqp.tile([P, BS], bf16, tag="sq_sq")
                nc.vector.tensor_mul(sq, dk, dk)
                nc.tensor.matmul(statB, ones_k, sq, start=(k == 0), stop=False)
            for t_src in (t3, s3):
                for k in range(HT):
                    sq = sqp.tile([P, BS], bf16, tag="sq_sq")
                    nc.vector.tensor_mul(sq, t_src[:, k, :], t_src[:, k, :])
                    nc.tensor.matmul(
                        statB, ones_k, sq,
                        start=False, stop=(t_src is s3 and k == HT - 1),
                    )
            colsum(statA, [t3, s3, abs3])
            negmu, rinvb = ln_rows(statA, statB, D3, f"g{j}", par=j % 2)
            return abs3, negmu, rinvb

        def gate_phase2(j, mj, abs3, negmu, rinvb):
            """Gate matmuls, mix, LN, blend for gate j."""
            is_audio = j == 3
            t_m = AUDIO if is_audio else mj
            t3 = tkw(t_m)
            s3 = omt if is_audio else tkw(AUDIO)

            # gate layer 1
            wge = wgae if is_audio else wgoe
            gps = [ppz.tile([P, BS], f32, tag="z", name=f"gps{mt}") for mt in range(MT)]
            if is_audio:
                parts = [(WGA, 0, t3), (WGA, 1, s3), (WGA, 2, abs3)]
            else:
                parts = [(WGO, 0, t3), (WGO, 2, abs3)]
            for mt in range(MT):
                cs = slice(mt * P, (mt + 1) * P)
                for pi, (WG, part, rhs3) in enumerate(parts):
                    w = wgp.tile([P, HT, P], bf16, tag="lin", name=f"wg{j}_{mt}_{part}")
                    nc.sync.dma_start(
                        out=w,
                        in_=WG.ap()[
                            part * HT : (part + 1) * HT, :, mt * P : (mt + 1) * P
                        ].rearrange("k p c -> p k c"),
                    )
                    for k in range(HT):
                        nc.tensor.matmul(
                            gps[mt], w[:, k, :], rhs3[:, k, :],
                            start=(pi == 0 and k == 0), stop=False,
                        )
                nc.tensor.matmul(
                    gps[mt], wge[0:1, cs], negmu, start=False, stop=True,
                )
            rb = bcast(rinvb, "rb")
            cb_off = 4 if is_audio else 0
            col = 1 if is_audio else 0
            gp = pps.tile([1, BS], f32, tag="statA", name=f"gp{j}")
            for mt in range(MT):
                hm = wk.tile([P, BS], f32, tag="hm")
                if is_audio:
                    nc.vector.tensor_mul(hm, gps[mt], rb)
                else:
                    nc.vector.tensor_add(hm, gps[mt], S[:, mt, :])
                    nc.vector.tensor_mul(hm, hm, rb)
                hg1 = wk.tile([P, BS], bf16, tag="hg", name=f"hg{mt}")
                nc.scalar.activation(
                    hg1, hm, AF.Gelu,
                    bias=cbt[:, cb_off + mt : cb_off + mt + 1], scale=1.0,
                )
                nc.tensor.matmul(
                    gp, w2t[:, mt, col : col + 1], hg1,
                    start=(mt == 0), stop=(mt == MT - 1),
                )
            g_row = rowp.tile([1, BS], bf16, tag="g_row")
            nc.scalar.activation(
                g_row, gp, AF.Sigmoid, bias=sct[0:1, col : col + 1], scale=1.0,
            )
            gb = bcast(g_row, "gb")

            # pre = t + g * (a2o | o2a); LN over H; blend into tok in place
            src = mix_src["x"]
            pre = big.tile([P, HT, BS], bf16, tag="pre")
            nc.vector.tensor_mul(pre, src, b3(gb))
            nc.vector.tensor_add(flat(pre), flat(pre), flat(t3))
            stat2A = pps.tile([1, BS], f32, tag="statA", name=f"stat2A_{j}")
            stat2B = pps.tile([1, BS], f32, tag="statB", name=f"stat2B_{j}")
            colsum(stat2A, [pre])
            colsum_sq(stat2B, [pre])
            negmu2, rinvb2 = ln_rows(stat2A, stat2B, H, f"u{j}", par=j % 2)
            mb = bcast(negmu2, "mb")
            rb2 = bcast(rinvb2, "rb2")
            wcol = 16 if is_audio else 0
            bcol = 24 if is_audio else 8
            sm = am_t if is_audio else pv_t[j]
            bmf = ma_t if is_audio else mo_t[j]
            # whole-token LN apply + blend: tok = bmf * (sm ? ln(pre) : t)
            q3 = big.tile([P, HT, BS], bf16, tag="q3", name=f"q3_{j}")
            nc.vector.tensor_add(q3, pre, b3(mb))
            nc.vector.tensor_mul(q3, q3, b3(rb2))
            for kt in range(HT):
                nc.vector.tensor_scalar(
                    q3[:, kt, :], q3[:, kt, :],
                    lnv[:, wcol + kt : wcol + kt + 1], lnv[:, bcol + kt : bcol + kt + 1],
                    op0=OP.mult, op1=OP.add,
                )
            nc.vector.copy_predicated(t3, b3(sm), q3)
            nc.vector.tensor_mul(t3, t3, b3(bmf))
            if not is_audio:
                if j == 0:
                    nc.vector.tensor_mul(omt, t3, b3(cm_t[j]))
                else:
                    tmp3 = big.tile([P, HT, BS], bf16, tag="q3", name=f"omtmp_{j}")
                    nc.vector.tensor_mul(tmp3, t3, b3(cm_t[j]))
                    nc.vector.tensor_add(omt, omt, tmp3)

        # LMF chain-index order: factors stored with M reordered as MORD so
        # the audio (blended last) chain comes last; r=0 partial products
        # for the non-audio tokens are emitted between mixer phases to keep
        # the PE busy during the gates' serial post-chains.
        MORD = (0, 2, 3, 1)
        uvt_l = [uvt[m] for m in MORD]
        acc = big.tile([P, HT, BS], f32r, tag="acc")
        soth = big.tile([P, HT, BS], bf16, tag="soth")

        def lmf_r0_chain(mi):
            for ht in range(HT):
                ft = ftp.tile([P, 9, P], bf16, tag="ft0")
                nc.sync.dma_start(out=ft, in_=FT.ap()[0, ht, mi])
                zp = ppz.tile([P, BS], f32, tag="z")
                for kt in range(HT):
                    nc.tensor.matmul(
                        zp, ft[:, kt, :], tk(MORD[mi], kt), start=(kt == 0), stop=False
                    )
                nc.tensor.matmul(zp, ft[0:2, 8, :], uvt_l[mi], start=False, stop=True)
                if mi == 0:
                    nc.scalar.activation(soth[:, ht, :], zp, AF.Copy)
                elif mi < 3:
                    nc.vector.tensor_mul(soth[:, ht, :], soth[:, ht, :], zp)
                else:
                    nc.vector.tensor_mul(acc[:, ht, :], soth[:, ht, :], zp)

        # software-pipelined emission: gate j+1's stats overlap gate j's
        # post-matmul chain on the PE; LMF r=0 chains fill blend windows.
        p1 = {}
        p1[0] = gate_phase1(0, OTHERS[0])
        p1[1] = gate_phase1(1, OTHERS[1])
        gate_phase2(0, OTHERS[0], *p1[0])
        p1[2] = gate_phase1(2, OTHERS[2])
        gate_phase2(1, OTHERS[1], *p1[1])
        gate_phase2(2, OTHERS[2], *p1[2])
        lmf_r0_chain(0)
        lmf_r0_chain(1)

        # ---- o2a = others_mean @ o2a_w.T ----
        o2ar = big.tile([P, HT, BS], bf16, tag="axr", name="o2ar")
        linmap(O2AT, omt, o2ar)
        mix_src["x"] = o2ar

        p1[3] = gate_phase1(3, AUDIO)
        lmf_r0_chain(2)
        gate_phase2(3, AUDIO, *p1[3])
        lmf_r0_chain(3)

        # ---- LMF ranks 1..R-1; LN1 stats interleaved into the last rank ----
        stat3A = pps.tile([1, BS], f32, tag="statA", name="stat3A")
        stat3B = pps.tile([1, BS], f32, tag="statB", name="stat3B")

        def stat3_for(ht, start, stop):
            nc.vector.tensor_scalar_add(
                acc[:, ht, :], acc[:, ht, :], lnv[:, 72 + ht : 72 + ht + 1]
            )
            sq = sqp.tile([P, BS], f32r, tag="sq_f")
            nc.vector.tensor_mul(sq, acc[:, ht, :], acc[:, ht, :])
            nc.tensor.matmul(stat3A, ones_kf, acc[:, ht, :], start=start, stop=stop)
            nc.tensor.matmul(stat3B, ones_kf, sq, start=start, stop=stop)

        for r in range(1, R):
            last = r == R - 1
            for ht in range(HT):
                ft = ftp.tile([P, M, 9, P], bf16, tag="ft")
                nc.sync.dma_start(
                    out=ft, in_=FT.ap()[r, ht].rearrange("m p k c -> p m k c")
                )
                zps = []
                for m in range(M):
                    zp = ppz.tile([P, BS], f32, tag="z")
                    for kt in range(HT):
                        nc.tensor.matmul(
                            zp, ft[:, m, kt, :], tk(MORD[m], kt),
                            start=(kt == 0), stop=False,
                        )
                    nc.tensor.matmul(zp, ft[0:2, m, 8, :], uvt_l[m], start=False, stop=True)
                    zps.append(zp)
                s0 = wk.tile([P, BS], f32, tag="s0")
                nc.scalar.activation(s0, zps[0], AF.Copy)
                nc.vector.tensor_mul(s0, s0, zps[1])
                nc.vector.tensor_mul(s0, s0, zps[2])
                nc.vector.tensor_mul(s0, s0, zps[3])
                nc.vector.tensor_add(acc[:, ht, :], acc[:, ht, :], s0)
                if last and ht >= 1:
                    stat3_for(ht - 1, start=(ht == 1), stop=False)
        stat3_for(HT - 1, start=False, stop=True)
        negmu3, rinvb3 = ln_rows(stat3A, stat3B, H, "l1", hi_mu=True)
        mb3 = bcast(negmu3, "mbf", dt=f32)
        rb3 = bcast(rinvb3, "rb2")
        h1 = big.tile([P, HT, BS], bf16, tag="pre", name="h1")
        nc.vector.tensor_add(h1, acc, b3(mb3))
        nc.vector.tensor_mul(h1, h1, b3(rb3))
        for kt in range(HT):
            nc.vector.tensor_scalar(
                h1[:, kt, :], h1[:, kt, :],
                lnv[:, 32 + kt : 32 + kt + 1], lnv[:, 40 + kt : 40 + kt + 1],
                op0=OP.mult, op1=OP.add,
            )

        # h2 = gelu(h1 @ out_w.T + out_b); LN2; write out
        h2 = big.tile([P, HT, BS], bf16, tag="abs0", name="h2")
        for ho in range(HT):
            wt = wgp.tile([P, HT, P], bf16, tag="lin", name=f"ow{ho}")
            nc.sync.dma_start(out=wt, in_=OUTWT.ap()[ho].rearrange("k p c -> p k c"))
            ps = ppz.tile([P, BS], f32, tag="z")
            for kt in range(HT):
                nc.tensor.matmul(
                    ps, wt[:, kt, :], h1[:, kt, :],
                    start=(kt == 0), stop=(kt == HT - 1),
                )
            nc.scalar.activation(
                h2[:, ho, :], ps, AF.Gelu, bias=lnv[:, 64 + ho : 64 + ho + 1], scale=1.0
            )
        stat4A = pps.tile([1, BS], f32, tag="statA", name="stat4A")
        stat4B = pps.tile([1, BS], f32, tag="statB", name="stat4B")
        colsum(stat4A, [h2])
        colsum_sq(stat4B, [h2])
        negmu4, rinvb4 = ln_rows(stat4A, stat4B, H, "l2", hi_mu=True)
        mb4 = bcast(negmu4, "mbf", dt=f32)
        rb4 = bcast(rinvb4, "rb2")
        for kt in range(HT):
            fin = wk.tile([P, BS], f32, tag="fin")
            nc.vector.tensor_add(fin, h2[:, kt, :], mb4)
            nc.vector.tensor_mul(fin, fin, rb4)
            nc.vector.tensor_scalar(
                fin, fin, lnv[:, 48 + kt : 48 + kt + 1], lnv[:, 56 + kt : 56 + kt + 1],
                op0=OP.mult, op1=OP.add,
            )
            nc.sync.dma_start(out=OUT.ap()[kt * P : (kt + 1) * P, :], in_=fin)

    nc.compile()
    return nc


def _host_prep(inputs):
    tokens = np.asarray(inputs["tokens"], np.float32)
    token_mask = np.asarray(inputs["token_mask"])
    mask_f = token_mask.astype(np.float32)

    mo = mask_f[:, list(OTHERS)]                      # [B,3]
    ma = mask_f[:, AUDIO]                             # [B]
    pv = mo * ma[:, None]                             # [B,3]
    winv = (1.0 / np.clip(mo.sum(1), 1.0, None)).astype(np.float32)
    aum = ma * (mo.max(1) > 0)                        # [B]

    go_w1 = np.asarray(inputs["go_w1"], np.float32)
    ga_w1 = np.asarray(inputs["ga_w1"], np.float32)

    def gate_prep(w1, b1, lnw, lnb):
        W1w = w1 * lnw[None, :]                       # [MID, 3H]
        c1 = np.ascontiguousarray(W1w.sum(1).reshape(1, MID))
        cb = w1 @ lnb + b1                            # [MID]
        Wblocks = np.ascontiguousarray(W1w.T).reshape(3 * HT, P, MID)
        return Wblocks, c1, cb

    WGOv, c1go, cbgo = gate_prep(
        go_w1, np.asarray(inputs["go_b1"], np.float32),
        np.asarray(inputs["ln_go_w"], np.float32), np.asarray(inputs["ln_go_b"], np.float32),
    )
    WGAv, c1ga, cbga = gate_prep(
        ga_w1, np.asarray(inputs["ga_b1"], np.float32),
        np.asarray(inputs["ln_ga_w"], np.float32), np.asarray(inputs["ln_ga_b"], np.float32),
    )
    CBv = np.ascontiguousarray(
        np.concatenate([cbgo.reshape(MT, P).T, cbga.reshape(MT, P).T], axis=1)
    ).astype(np.float32)                              # [P, 8]
    W2v = np.stack(
        [np.asarray(inputs["go_w2"], np.float32).reshape(MID),
         np.asarray(inputs["ga_w2"], np.float32).reshape(MID)], axis=1
    )                                                 # [MID, 2]
    W2v = np.ascontiguousarray(W2v.reshape(MT, P, 2).transpose(1, 0, 2))
    SCv = np.zeros((1, 8), np.float32)
    SCv[0, 0] = np.asarray(inputs["go_b2"], np.float32).reshape(-1)[0]
    SCv[0, 1] = np.asarray(inputs["ga_b2"], np.float32).reshape(-1)[0]
    SCv[0, 2] = EPS

    def tile_blocks(w):
        wt = np.ascontiguousarray(np.asarray(w, np.float32).T)    # [H_in, H_out]
        return np.ascontiguousarray(
            wt.reshape(HT, P, HT, P).transpose(2, 0, 1, 3)
        ).astype(bfnp)

    A2OTv = tile_blocks(inputs["a2o_w"])
    O2ATv = tile_blocks(inputs["o2a_w"])
    OUTWTv = tile_blocks(inputs["out_w"])

    def cols(name):
        return np.asarray(inputs[name], np.float32).reshape(HT, P).T

    LNVv = np.zeros((P, 80), np.float32)
    for i, name in enumerate(
        ["ln_o_w", "ln_o_b", "ln_a_w", "ln_a_b", "out_ln1_w", "out_ln1_b",
         "out_ln2_w", "out_ln2_b", "out_b", "lmf_bias"]
    ):
        LNVv[:, 8 * i : 8 * (i + 1)] = cols(name)

    factors = np.asarray(inputs["factors"], np.float32)
    rank_w = np.asarray(inputs["rank_w"], np.float32)
    Ff = factors.copy()
    Ff[AUDIO] = Ff[AUDIO] * rank_w[:, None, None]
    # partition-major layout [R, HT, M, P, 9, P]
    FTv = np.zeros((R, HT, M, P, 9, P), np.float32)
    main = Ff[:, :, 1:, :].reshape(M, R, HT, P, HT, P)   # [m, r, kt, pk, ht, ph]
    FTv[:, :, :, :, :8, :] = main.transpose(1, 4, 0, 3, 2, 5)
    bias = Ff[:, :, 0, :].reshape(M, R, HT, P)           # [m, r, ht, ph]
    FTv[:, :, :, 0, 8, :] = bias.transpose(1, 2, 0, 3)
    ones_row = np.ones((R, HT, M, P), np.float32)
    ones_row[:, :, AUDIO, :] = rank_w[:, None, None]
    FTv[:, :, :, 1, 8, :] = ones_row
    # reorder M to the kernel's chain order (audio last)
    FTv = np.ascontiguousarray(FTv[:, :, [0, 2, 3, 1]]).astype(bfnp)

    shared = dict(
        WGO=WGOv.astype(bfnp), WGA=WGAv.astype(bfnp),
        WGOe=c1go.astype(bfnp), WGAe=c1ga.astype(bfnp),
        W2=W2v.astype(bfnp), CB=CBv, SC=SCv,
        A2OT=A2OTv, O2AT=O2ATv, OUTWT=OUTWTv, LNV=LNVv, FT=FTv,
    )

    in_maps = []
    for c in range(NCORES):
        sl = slice(c * BS, (c + 1) * BS)
        tokTv = np.ascontiguousarray(tokens[sl].transpose(1, 2, 0)).astype(bfnp)
        u8v = np.zeros((4, BS), np.uint8)
        u8v[0:3] = pv[sl].T > 0
        u8v[3] = aum[sl] > 0
        f16v = np.zeros((7, BS), np.float32)
        f16v[0:3] = mo[sl].T
        f16v[3] = ma[sl]
        f16v[4:7] = (mo[sl] * winv[sl, None]).T
        uvv = np.zeros((M, 2, BS), np.float32)
        uvv[:, 0, :] = mask_f[sl].T
        uvv[:, 1, :] = 1.0 - mask_f[sl].T
        in_maps.append(dict(
            tokT=tokTv, u8rows=u8v, f16rows=f16v.astype(bfnp),
            uv=uvv.astype(bfnp), **shared,
        ))
    return in_maps


def kernel(**inputs):
    global _cached_nc, LAST_RESULTS
    if _cached_nc is None:
        _cached_nc = _build()
    in_maps = _host_prep(inputs)
    res = run_bass_kernel_spmd(
        _cached_nc, in_maps, core_ids=list(range(NCORES)), trace=TRACE
    )
    LAST_RESULTS = res
    out = np.stack([res.results[c]["outT"].T for c in range(NCORES)], axis=0)
    return np.ascontiguousarray(out.reshape(B, H)).astype(np.float32)


# revision 29
# speedup vs baseline: 2.0930x; 1.0081x over previous
"""TRN2 Bass kernel for nn_BlendEmoBackbone: gated audio mixer + low-rank
multiplicative fusion, data-parallel over batch on 8 NeuronCores.

Strategy (v2, bf16):
- Pure data parallel: each core handles B/8 = 512 batch rows; gate MLP
  weights and LMF factor tensors replicated (bf16 halves HBM traffic).
- All activations in transposed [feature, batch] layout; every matmul
  contracts over the partition dim. bf16 operands stream 1 cycle/row on
  the PE (fp32/f32r streams at ~2 cycles/row on real TRN2).
- LayerNorm stats via PE ones-matmul column sums; -mu folded into gate
  matmuls as an extra K=1 row.
- LMF where(mask, z, 1) + x_aug ones-column folded into a K=2 tail tile
  in the same psum chain; rank_w folded into the audio factor slices.
- Factors stored partition-major [R,HT,M,P,9,P] so each (r,ht) loads
  with ONE contiguous-per-partition DMA (2.3KB lines).
- WGO gate weights resident in SBUF (single DMA, reused by 3 gates);
  the audio-source half of the other-gate matmul computed once (S).
- Row->tile broadcasts on gpsimd (partition_broadcast); abs/gelu/
  sigmoid/psum-copies on the scalar engine; products/blends on DVE in
  bf16 where precision allows.
"""

import numpy as np
import ml_dtypes
from contextlib import ExitStack

import concourse.bass as bass
from concourse import bacc
import concourse.tile as tile
from concourse import mybir
from concourse.bass_utils import run_bass_kernel_spmd

B, M, H, R = 4096, 4, 1024, 10
NCORES = 8
BS = B // NCORES          # 512 batch rows per core
MID = 512
P = 128
HT = H // P               # 8 h-tiles
MT = MID // P             # 4 mid-tiles
D3 = 3 * H
OTHERS = (0, 2, 3)
AUDIO = 1
EPS = 1e-5

f32 = mybir.dt.float32
f32r = mybir.dt.float32r
bf16 = mybir.dt.bfloat16
u8 = mybir.dt.uint8
AF = mybir.ActivationFunctionType
OP = mybir.AluOpType
bfnp = ml_dtypes.bfloat16

TRACE = False
LAST_RESULTS = None

_cached_nc = None


def _build():
    nc = bacc.Bacc("TRN2", target_bir_lowering=False, debug=False)

    # ---- DRAM parameters (per core) ----
    tokT = nc.declare_dram_parameter("tokT", [M, H, BS], bf16, isOutput=False)
    # u8 rows: 0-2 pv_j, 3 am(aum)
    u8rows = nc.declare_dram_parameter("u8rows", [4, BS], u8, isOutput=False)
    # bf16 rows: 0-2 mo_j, 3 ma, 4-6 cm_j
    f16rows = nc.declare_dram_parameter("f16rows", [7, BS], bf16, isOutput=False)
    uv = nc.declare_dram_parameter("uv", [M, 2, BS], bf16, isOutput=False)
    WGO = nc.declare_dram_parameter("WGO", [3 * HT, P, MID], bf16, isOutput=False)
    WGA = nc.declare_dram_parameter("WGA", [3 * HT, P, MID], bf16, isOutput=False)
    C1 = nc.declare_dram_parameter("C1", [P, MT, 2], f32, isOutput=False)
    W2 = nc.declare_dram_parameter("W2", [P, MT, 2], bf16, isOutput=False)
    CB = nc.declare_dram_parameter("CB", [P, 8], f32, isOutput=False)
    SC = nc.declare_dram_parameter("SC", [1, 8], f32, isOutput=False)
    # [ht_out, kt, P, P] tiled weight blocks (lhsT layout)
    A2OT = nc.declare_dram_parameter("A2OT", [HT, HT, P, P], bf16, isOutput=False)
    O2AT = nc.declare_dram_parameter("O2AT", [HT, HT, P, P], bf16, isOutput=False)
    OUTWT = nc.declare_dram_parameter("OUTWT", [HT, HT, P, P], bf16, isOutput=False)
    # cols: ln_o_w 0:8, ln_o_b 8:16, ln_a_w 16:24, ln_a_b 24:32,
    #       ln1w 32:40, ln1b 40:48, ln2w 48:56, ln2b 56:64, outb 64:72, lmfb 72:80
    LNV = nc.declare_dram_parameter("LNV", [P, 80], f32, isOutput=False)
    # partition-major factor blocks; [.., p, 0:8, :] = main k-tiles,
    # [.., 0:2, 8, :] = [bias_row; ones_or_rankw_row]
    FT = nc.declare_dram_parameter("FT", [R, HT, M, P, 9, P], bf16, isOutput=False)
    OUT = nc.declare_dram_parameter("outT", [H, BS], bf16, isOutput=True)

    with tile.TileContext(nc) as tc, ExitStack() as ctx:
        kp = ctx.enter_context(tc.tile_pool(name="konst", bufs=1))
        tokp = ctx.enter_context(tc.tile_pool(name="tokp", bufs=1))
        big = ctx.enter_context(tc.tile_pool(name="big", bufs=1))
        wk = ctx.enter_context(tc.tile_pool(name="wk", bufs=2))
        bcp = ctx.enter_context(tc.tile_pool(name="bcp", bufs=1))
        sqp = ctx.enter_context(tc.tile_pool(name="sqp", bufs=9))
        sqf = ctx.enter_context(tc.tile_pool(name="sqf", bufs=3))
        wgp = ctx.enter_context(tc.tile_pool(name="wgp", bufs=2))
        ftp = ctx.enter_context(tc.tile_pool(name="ftp", bufs=2))
        rowp = ctx.enter_context(tc.tile_pool(name="rowp", bufs=1))
        ppz = ctx.enter_context(tc.tile_pool(name="ppz", bufs=2, space="PSUM"))
        ppg = ctx.enter_context(tc.tile_pool(name="ppg", bufs=4, space="PSUM"))
        pps = ctx.enter_context(tc.tile_pool(name="pps", bufs=1, space="PSUM"))

        # ---- constants / small loads ----
        ones_k = kp.tile([P, 1], bf16)
        nc.vector.memset(ones_k, 1.0)
        ones_kf32 = kp.tile([P, 1], f32)
        nc.vector.memset(ones_kf32, 1.0)
        ones_kf = ones_kf32.bitcast(f32r)

        def bc_row_dma(dst, src_ap):
            nc.sync.dma_start(
                out=dst,
                in_=bass.AP(
                    tensor=src_ap.tensor, offset=src_ap.offset, ap=[[0, P], [1, BS]]
                ),
            )

        u8t = []
        for i in range(4):
            t = kp.tile([P, BS], u8, tag=f"u8_{i}")
            bc_row_dma(t, u8rows.ap()[i : i + 1, :])
            u8t.append(t)
        pv_t, am_t = u8t[0:3], u8t[3]
        f16t = []
        for i in range(7):
            t = kp.tile([P, BS], bf16, tag=f"f16_{i}")
            bc_row_dma(t, f16rows.ap()[i : i + 1, :])
            f16t.append(t)
        mo_t, ma_t, cm_t = f16t[0:3], f16t[3], f16t[4:7]
        uvt = []
        for m in range(M):
            t = kp.tile([2, BS], bf16, tag=f"uv_{m}")
            nc.sync.dma_start(out=t, in_=uv.ap()[m])
            uvt.append(t)
        cbt = kp.tile([P, 8], f32)
        nc.sync.dma_start(out=cbt, in_=CB.ap())
        sct = kp.tile([1, 8], f32)
        nc.sync.dma_start(out=sct, in_=SC.ap())
        lnv = kp.tile([P, 80], f32)
        nc.sync.dma_start(out=lnv, in_=LNV.ap())
        w2t = kp.tile([P, MT, 2], bf16)
        nc.sync.dma_start(out=w2t, in_=W2.ap())
        c1t = kp.tile([P, MT, 2], f32)
        nc.sync.dma_start(out=c1t, in_=C1.ap())



        # ---- tokens (transposed, bf16) ----
        tok = tokp.tile([P, M, HT, BS], bf16)
        for m in range(M):
            nc.sync.dma_start(
                out=tok[:, m], in_=tokT.ap()[m].rearrange("(ht p) b -> p ht b", p=P)
            )

        def tk(m, kt):
            return tok[:, m, kt, :]

        def tkw(m):  # whole-token [P, HT, BS] view
            return tok[:, m]

        def flat(t3):
            return t3.rearrange("p a b -> p (a b)")

        def b3(t2):  # [P,BS] -> broadcast [P,HT,BS]
            return t2.unsqueeze(1).broadcast_to([P, HT, BS])

        # ---- helpers ----
        def ln_rows(statA, statB, n, tag, hi_mu=False, par=0):
            mdt = f32 if hi_mu else bf16
            mtag = "negmuf" if hi_mu else f"negmu{par}"
            negmu = rowp.tile([1, BS], mdt, tag=mtag, name=f"negmu_{tag}")
            nc.scalar.activation(negmu, statA, AF.Copy, bias=0.0, scale=-1.0 / n)
            ex2 = rowp.tile([1, BS], f32, tag="ex2", name=f"ex2_{tag}")
            nc.scalar.activation(ex2, statB, AF.Copy, bias=0.0, scale=1.0 / n)
            msq = rowp.tile([1, BS], f32, tag="msq", name=f"msq_{tag}")
            nc.scalar.activation(msq, negmu, AF.Square)
            nc.vector.tensor_sub(ex2, ex2, msq)                      # var in place
            nc.vector.tensor_scalar_max(ex2, ex2, 0.0)               # bf16 rounding guard
            nc.scalar.activation(msq, ex2, AF.Sqrt, bias=sct[0:1, 2:3], scale=1.0)
            nc.vector.reciprocal(ex2, msq)
            rinvb = rowp.tile([1, BS], bf16, tag=f"rinvb{par}", name=f"rinvb_{tag}")
            nc.scalar.activation(rinvb, ex2, AF.Copy)
            return negmu, rinvb

        def bcast(row, tag, dt=bf16):
            """Broadcast a [1,BS] row to [P,BS] via gpsimd."""
            sb = bcp.tile([P, BS], dt, tag=f"bc_{tag}")
            nc.gpsimd.partition_broadcast(sb, row)
            return sb

        def colsum_sq(statB, tiles3, dt=bf16, name="sq"):
            """statB += per-column sums of squares of all kt slices."""
            chunks = []
            for t3 in tiles3:
                for kt in range(HT):
                    chunks.append(t3[:, kt, :])
            n = len(chunks)
            lhs = ones_k if dt == bf16 else ones_kf
            for i, ch in enumerate(chunks):
                sq = sqp.tile([P, BS], dt, tag=f"sq_{name}")
                nc.vector.tensor_mul(sq, ch, ch)
                nc.tensor.matmul(statB, lhs, sq, start=(i == 0), stop=(i == n - 1))

        def colsum(statA, tiles3, dt=bf16):
            chunks = []
            for t3 in tiles3:
                for kt in range(HT):
                    chunks.append(t3[:, kt, :])
            n = len(chunks)
            lhs = ones_k if dt == bf16 else ones_kf
            for i, ch in enumerate(chunks):
                nc.tensor.matmul(statA, lhs, ch, start=(i == 0), stop=(i == n - 1))

        def linmap(WT, src3, dst3):
            """dst3[ho] = sum_kt WT[ho,kt].T @ src3[kt]; WT streamed from DRAM."""
            for ho in range(HT):
                wt = wgp.tile([P, HT, P], bf16, tag="lin")
                nc.sync.dma_start(out=wt, in_=WT.ap()[ho].rearrange("k p c -> p k c"))
                ps = ppz.tile([P, BS], f32, tag="z")
                for kt in range(HT):
                    nc.tensor.matmul(
                        ps, wt[:, kt, :], src3[:, kt, :],
                        start=(kt == 0), stop=(kt == HT - 1),
                    )
                nc.scalar.activation(dst3[:, ho, :], ps, AF.Copy)

        # ---- a2o = audio @ a2o_w.T, in T layout (bf16) ----
        a2or = big.tile([P, HT, BS], bf16, tag="axr")
        linmap(A2OT, tkw(AUDIO), a2or)

        # S_mt = sum_k Wgo_s[k].T @ audio  (shared source half of gate1)
        S = big.tile([P, MT, BS], bf16, tag="Sg")
        for mt in range(MT):
            wS = wgp.tile([P, HT, P], bf16, tag="lin", name=f"wS{mt}")
            nc.sync.dma_start(
                out=wS,
                in_=WGO.ap()[HT : 2 * HT, :, mt * P : (mt + 1) * P].rearrange(
                    "k p c -> p k c"
                ),
            )
            ps = ppz.tile([P, BS], f32, tag="z")
            for k in range(HT):
                nc.tensor.matmul(
                    ps, wS[:, k, :], tk(AUDIO, k),
                    start=(k == 0), stop=(k == HT - 1),
                )
            nc.scalar.activation(S[:, mt, :], ps, AF.Copy)

        omt = big.tile([P, HT, BS], bf16, tag="om")  # others_mean accumulator
        mix_src = {"x": a2or}

        def gate_phase1(j, mj):
            """Stats + gate1 matmuls + LN rows for gate j. Emission order
            keeps the PE fed: gate1 halves (no stats dependency) are
            interleaved with the DVE-paced stat chains."""
            is_audio = j == 3
            t_m = AUDIO if is_audio else mj
            t3 = tkw(t_m)
            s3 = omt if is_audio else tkw(AUDIO)

            abs3 = big.tile([P, HT, BS], bf16, tag="abs", name=f"abs3_{j}")
            statA = pps.tile([1, BS], f32, tag="statA", name=f"statA_{j}")
            statB = pps.tile([1, BS], f32, tag="statB", name=f"statB_{j}")
            gps = [
                ppg.tile([P, BS], f32, tag="g", name=f"gps{j}_{mt}")
                for mt in range(MT)
            ]
            if is_audio:
                parts = [(WGA, 0, t3), (WGA, 1, s3), (WGA, 2, abs3)]
            else:
                parts = [(WGO, 0, t3), (WGO, 2, abs3)]

            # DVE production first: d, |d| (ACT), d^2 tiles
            dsq = []
            for k in range(HT):
                dk = wk.tile([P, BS], bf16, tag="dk")
                nc.vector.tensor_sub(dk, t3[:, k, :], s3[:, k, :])
                nc.scalar.activation(abs3[:, k, :], dk, AF.Abs)
                sq = sqp.tile([P, BS], bf16, tag="sq_sq")
                nc.vector.tensor_mul(sq, dk, dk)
                dsq.append(sq)

            def mt_chain(mt):
                for pi, (WG, part, rhs3) in enumerate(parts):
                    w = wgp.tile([P, HT, P], bf16, tag="lin", name=f"wg{j}_{mt}_{part}")
                    nc.sync.dma_start(
                        out=w,
                        in_=WG.ap()[
                            part * HT : (part + 1) * HT, :, mt * P : (mt + 1) * P
                        ].rearrange("k p c -> p k c"),
                    )
                    for k in range(HT):
                        nc.tensor.matmul(
                            gps[mt], w[:, k, :], rhs3[:, k, :],
                            start=(pi == 0 and k == 0),
                            stop=(pi == len(parts) - 1 and k == HT - 1),
                        )

            mt_chain(0)
            mt_chain(1)
            for i, sq in enumerate(dsq):
                nc.tensor.matmul(statB, ones_k, sq, start=(i == 0), stop=False)
            for t_src in (t3, s3):
                for k in range(HT):
                    sq = sqp.tile([P, BS], bf16, tag="sq_sq")
                    nc.vector.tensor_mul(sq, t_src[:, k, :], t_src[:, k, :])
                    nc.tensor.matmul(
                        statB, ones_k, sq,
                        start=False, stop=(t_src is s3 and k == HT - 1),
                    )
            mt_chain(2)
            colsum(statA, [t3, s3, abs3])
            mt_chain(3)
            negmu, rinvb = ln_rows(statA, statB, D3, f"g{j}", par=j % 2)
            return abs3, negmu, rinvb, gps

        def gate_phase2a(j, mj, abs3, negmu, rinvb, gps):
            """Gate layer 2 + mix pre-activation for gate j."""
            is_audio = j == 3
            t3 = tkw(AUDIO if is_audio else mj)
            rb = bcast(rinvb, "rb")
            nmb = bcast(negmu, "nm")
            cb_off = 4 if is_audio else 0
            col = 1 if is_audio else 0
            gp = pps.tile([1, BS], f32, tag="statA", name=f"gp{j}")
            for mt in range(MT):
                hm = wk.tile([P, BS], f32, tag="hm")
                # hm = gps + (-mu)*c1 [+ S]; then * rinv
                nc.vector.scalar_tensor_tensor(
                    hm, nmb, c1t[:, mt, col : col + 1], gps[mt],
                    op0=OP.mult, op1=OP.add,
                )
                if not is_audio:
                    nc.vector.tensor_add(hm, hm, S[:, mt, :])
                nc.vector.tensor_mul(hm, hm, rb)
                hg1 = wk.tile([P, BS], bf16, tag="hg", name=f"hg{mt}")
                nc.scalar.activation(
                    hg1, hm, AF.Gelu,
                    bias=cbt[:, cb_off + mt : cb_off + mt + 1], scale=1.0,
                )
                nc.tensor.matmul(
                    gp, w2t[:, mt, col : col + 1], hg1,
                    start=(mt == 0), stop=(mt == MT - 1),
                )
            g_row = rowp.tile([1, BS], bf16, tag="g_row")
            nc.scalar.activation(
                g_row, gp, AF.Sigmoid, bias=sct[0:1, col : col + 1], scale=1.0,
            )
            gb = bcast(g_row, "gb")
            # pre = t + g * (a2o | o2a)
            src = mix_src["x"]
            pre = big.tile([P, HT, BS], bf16, tag=f"pre{j % 2}", name=f"pre{j}")
            nc.vector.tensor_mul(pre, src, b3(gb))
            nc.vector.tensor_add(flat(pre), flat(pre), flat(t3))
            return pre

        def gate_phase2b(j, mj, pre):
            """Mix LN + blend for gate j."""
            is_audio = j == 3
            t_m = AUDIO if is_audio else mj
            t3 = tkw(t_m)
            stat2A = pps.tile([1, BS], f32, tag="statA", name=f"stat2A_{j}")
            stat2B = pps.tile([1, BS], f32, tag="statB", name=f"stat2B_{j}")
            colsum(stat2A, [pre])
            colsum_sq(stat2B, [pre])
            negmu2, rinvb2 = ln_rows(stat2A, stat2B, H, f"u{j}", par=2 + (j % 2))
            mb = bcast(negmu2, "mb")
            rb2 = bcast(rinvb2, "rb2")
            wcol = 16 if is_audio else 0
            bcol = 24 if is_audio else 8
            sm = am_t if is_audio else pv_t[j]
            bmf = ma_t if is_audio else mo_t[j]
            # whole-token LN apply + blend: tok = bmf * (sm ? ln(pre) : t)
            q3 = big.tile([P, HT, BS], bf16, tag="q3", name=f"q3_{j}")
            nc.vector.tensor_add(q3, pre, b3(mb))
            nc.vector.tensor_mul(q3, q3, b3(rb2))
            for kt in range(HT):
                nc.vector.tensor_scalar(
                    q3[:, kt, :], q3[:, kt, :],
                    lnv[:, wcol + kt : wcol + kt + 1], lnv[:, bcol + kt : bcol + kt + 1],
                    op0=OP.mult, op1=OP.add,
                )
            nc.vector.copy_predicated(t3, b3(sm), q3)
            nc.vector.tensor_mul(t3, t3, b3(bmf))
            if not is_audio:
                if j == 0:
                    nc.vector.tensor_mul(omt, t3, b3(cm_t[j]))
                else:
                    tmp3 = big.tile([P, HT, BS], bf16, tag="q3", name=f"omtmp_{j}")
                    nc.vector.tensor_mul(tmp3, t3, b3(cm_t[j]))
                    nc.vector.tensor_add(omt, omt, tmp3)

        # LMF chain-index order: factors stored with M reordered as MORD so
        # the audio (blended last) chain comes last; r=0 partial products
        # for the non-audio tokens are emitted between mixer phases to keep
        # the PE busy during the gates' serial post-chains.
        MORD = (0, 2, 3, 1)
        uvt_l = [uvt[m] for m in MORD]
        acc = big.tile([P, HT, BS], f32r, tag="acc")
        soth = big.tile([P, HT, BS], bf16, tag="soth")

        def lmf_r0_chain(mi):
            for ht in range(HT):
                ft = ftp.tile([P, 9, P], bf16, tag="ft0")
                nc.sync.dma_start(out=ft, in_=FT.ap()[0, ht, mi])
                zp = ppz.tile([P, BS], f32, tag="z")
                for kt in range(HT):
                    nc.tensor.matmul(
                        zp, ft[:, kt, :], tk(MORD[mi], kt), start=(kt == 0), stop=False
                    )
                nc.tensor.matmul(zp, ft[0:2, 8, :], uvt_l[mi], start=False, stop=True)
                if mi == 0:
                    nc.scalar.activation(soth[:, ht, :], zp, AF.Copy)
                elif mi < 3:
                    nc.vector.tensor_mul(soth[:, ht, :], soth[:, ht, :], zp)
                else:
                    nc.vector.tensor_mul(acc[:, ht, :], soth[:, ht, :], zp)

        # software-pipelined emission: gate j+1's stats+gate1 overlap gate
        # j's post-matmul chain on the PE; LMF r=0 chains fill blend windows.
        p1, p2 = {}, {}
        p1[0] = gate_phase1(0, OTHERS[0])
        p1[1] = gate_phase1(1, OTHERS[1])
        p2[0] = gate_phase2a(0, OTHERS[0], *p1[0])
        p1[2] = gate_phase1(2, OTHERS[2])
        p2[1] = gate_phase2a(1, OTHERS[1], *p1[1])
        gate_phase2b(0, OTHERS[0], p2[0])
        p2[2] = gate_phase2a(2, OTHERS[2], *p1[2])
        gate_phase2b(1, OTHERS[1], p2[1])
        gate_phase2b(2, OTHERS[2], p2[2])
        lmf_r0_chain(0)
        lmf_r0_chain(1)

        # ---- o2a = others_mean @ o2a_w.T ----
        o2ar = big.tile([P, HT, BS], bf16, tag="axr", name="o2ar")
        linmap(O2AT, omt, o2ar)
        mix_src["x"] = o2ar

        p1[3] = gate_phase1(3, AUDIO)
        lmf_r0_chain(2)
        p2[3] = gate_phase2a(3, AUDIO, *p1[3])
        gate_phase2b(3, AUDIO, p2[3])
        lmf_r0_chain(3)

        # ---- LMF ranks 1..R-1; LN1 stats interleaved into the last rank ----
        stat3A = pps.tile([1, BS], f32, tag="statA", name="stat3A")
        stat3B = pps.tile([1, BS], f32, tag="statB", name="stat3B")

        def stat3_for(ht, start, stop):
            nc.vector.tensor_scalar_add(
                acc[:, ht, :], acc[:, ht, :], lnv[:, 72 + ht : 72 + ht + 1]
            )
            sq = sqf.tile([P, BS], f32r, tag="sq_f")
            nc.vector.tensor_mul(sq, acc[:, ht, :], acc[:, ht, :])
            nc.tensor.matmul(stat3A, ones_kf, acc[:, ht, :], start=start, stop=stop)
            nc.tensor.matmul(stat3B, ones_kf, sq, start=start, stop=stop)

        for r in range(1, R):
            last = r == R - 1
            for ht in range(HT):
                ft = ftp.tile([P, M, 9, P], bf16, tag="ft")
                nc.sync.dma_start(
                    out=ft, in_=FT.ap()[r, ht].rearrange("m p k c -> p m k c")
                )
                zps = []
                for m in range(M):
                    zp = ppz.tile([P, BS], f32, tag="z")
                    for kt in range(HT):
                        nc.tensor.matmul(
                            zp, ft[:, m, kt, :], tk(MORD[m], kt),
                            start=(kt == 0), stop=False,
                        )
                    nc.tensor.matmul(zp, ft[0:2, m, 8, :], uvt_l[m], start=False, stop=True)
                    zps.append(zp)
                s0 = wk.tile([P, BS], f32, tag="s0")
                nc.scalar.activation(s0, zps[0], AF.Copy)
                nc.vector.tensor_mul(s0, s0, zps[1])
                nc.vector.tensor_mul(s0, s0, zps[2])
                nc.vector.tensor_mul(s0, s0, zps[3])
                nc.vector.tensor_add(acc[:, ht, :], acc[:, ht, :], s0)
                if last and ht >= 1:
                    stat3_for(ht - 1, start=(ht == 1), stop=False)
        stat3_for(HT - 1, start=False, stop=True)
        negmu3, rinvb3 = ln_rows(stat3A, stat3B, H, "l1", hi_mu=True)
        mb3 = bcast(negmu3, "mbf", dt=f32)
        rb3 = bcast(rinvb3, "rb2")
        h1 = big.tile([P, HT, BS], bf16, tag="pre", name="h1")
        nc.vector.tensor_add(h1, acc, b3(mb3))
        nc.vector.tensor_mul(h1, h1, b3(rb3))
        for kt in range(HT):
            nc.vector.tensor_scalar(
                h1[:, kt, :], h1[:, kt, :],
                lnv[:, 32 + kt : 32 + kt + 1], lnv[:, 40 + kt : 40 + kt + 1],
                op0=OP.mult, op1=OP.add,
            )

        # h2 = gelu(h1 @ out_w.T + out_b); LN2; write out
        h2 = big.tile([P, HT, BS], bf16, tag="abs", name="h2")
        for ho in range(HT):
            wt = wgp.tile([P, HT, P], bf16, tag="lin", name=f"ow{ho}")
            nc.sync.dma_start(out=wt, in_=OUTWT.ap()[ho].rearrange("k p c -> p k c"))
            ps = ppz.tile([P, BS], f32, tag="z")
            for kt in range(HT):
                nc.tensor.matmul(
                    ps, wt[:, kt, :], h1[:, kt, :],
                    start=(kt == 0), stop=(kt == HT - 1),
                )
            nc.scalar.activation(
                h2[:, ho, :], ps, AF.Gelu, bias=lnv[:, 64 + ho : 64 + ho + 1], scale=1.0
            )
        stat4A = pps.tile([1, BS], f32, tag="statA", name="stat4A")
        stat4B = pps.tile([1, BS], f32, tag="statB", name="stat4B")
        colsum(stat4A, [h2])
        colsum_sq(stat4B, [h2])
        negmu4, rinvb4 = ln_rows(stat4A, stat4B, H, "l2", hi_mu=True)
        mb4 = bcast(negmu4, "mbf", dt=f32)
        rb4 = bcast(rinvb4, "rb2")
        fin3 = big.tile([P, HT, BS], bf16, tag="q3", name="fin3")
        nc.vector.tensor_add(fin3, h2, b3(mb4))
        nc.vector.tensor_mul(fin3, fin3, b3(rb4))
        for kt in range(HT):
            nc.vector.tensor_scalar(
                fin3[:, kt, :], fin3[:, kt, :],
                lnv[:, 48 + kt : 48 + kt + 1], lnv[:, 56 + kt : 56 + kt + 1],
                op0=OP.mult, op1=OP.add,
            )
            nc.sync.dma_start(out=OUT.ap()[kt * P : (kt + 1) * P, :], in_=fin3[:, kt, :])

    nc.compile()
    return nc


def _host_prep(inputs):
    tokens = np.asarray(inputs["tokens"], np.float32)
    token_mask = np.asarray(inputs["token_mask"])
    mask_f = token_mask.astype(np.float32)

    mo = mask_f[:, list(OTHERS)]                      # [B,3]
    ma = mask_f[:, AUDIO]                             # [B]
    pv = mo * ma[:, None]                             # [B,3]
    winv = (1.0 / np.clip(mo.sum(1), 1.0, None)).astype(np.float32)
    aum = ma * (mo.max(1) > 0)                        # [B]

    go_w1 = np.asarray(inputs["go_w1"], np.float32)
    ga_w1 = np.asarray(inputs["ga_w1"], np.float32)

    def gate_prep(w1, b1, lnw, lnb):
        W1w = w1 * lnw[None, :]                       # [MID, 3H]
        c1 = np.ascontiguousarray(W1w.sum(1).reshape(1, MID))
        cb = w1 @ lnb + b1                            # [MID]
        Wblocks = np.ascontiguousarray(W1w.T).reshape(3 * HT, P, MID)
        return Wblocks, c1, cb

    WGOv, c1go, cbgo = gate_prep(
        go_w1, np.asarray(inputs["go_b1"], np.float32),
        np.asarray(inputs["ln_go_w"], np.float32), np.asarray(inputs["ln_go_b"], np.float32),
    )
    WGAv, c1ga, cbga = gate_prep(
        ga_w1, np.asarray(inputs["ga_b1"], np.float32),
        np.asarray(inputs["ln_ga_w"], np.float32), np.asarray(inputs["ln_ga_b"], np.float32),
    )
    CBv = np.ascontiguousarray(
        np.concatenate([cbgo.reshape(MT, P).T, cbga.reshape(MT, P).T], axis=1)
    ).astype(np.float32)                              # [P, 8]
    W2v = np.stack(
        [np.asarray(inputs["go_w2"], np.float32).reshape(MID),
         np.asarray(inputs["ga_w2"], np.float32).reshape(MID)], axis=1
    )                                                 # [MID, 2]
    W2v = np.ascontiguousarray(W2v.reshape(MT, P, 2).transpose(1, 0, 2))
    C1v = np.stack([c1go.reshape(MID), c1ga.reshape(MID)], axis=1)
    C1v = np.ascontiguousarray(C1v.reshape(MT, P, 2).transpose(1, 0, 2)).astype(np.float32)
    SCv = np.zeros((1, 8), np.float32)
    SCv[0, 0] = np.asarray(inputs["go_b2"], np.float32).reshape(-1)[0]
    SCv[0, 1] = np.asarray(inputs["ga_b2"], np.float32).reshape(-1)[0]
    SCv[0, 2] = EPS

    def tile_blocks(w):
        wt = np.ascontiguousarray(np.asarray(w, np.float32).T)    # [H_in, H_out]
        return np.ascontiguousarray(
            wt.reshape(HT, P, HT, P).transpose(2, 0, 1, 3)
        ).astype(bfnp)

    A2OTv = tile_blocks(inputs["a2o_w"])
    O2ATv = tile_blocks(inputs["o2a_w"])
    OUTWTv = tile_blocks(inputs["out_w"])

    def cols(name):
        return np.asarray(inputs[name], np.float32).reshape(HT, P).T

    LNVv = np.zeros((P, 80), np.float32)
    for i, name in enumerate(
        ["ln_o_w", "ln_o_b", "ln_a_w", "ln_a_b", "out_ln1_w", "out_ln1_b",
         "out_ln2_w", "out_ln2_b", "out_b", "lmf_bias"]
    ):
        LNVv[:, 8 * i : 8 * (i + 1)] = cols(name)

    factors = np.asarray(inputs["factors"], np.float32)
    rank_w = np.asarray(inputs["rank_w"], np.float32)
    Ff = factors.copy()
    Ff[AUDIO] = Ff[AUDIO] * rank_w[:, None, None]
    # partition-major layout [R, HT, M, P, 9, P]
    FTv = np.zeros((R, HT, M, P, 9, P), np.float32)
    main = Ff[:, :, 1:, :].reshape(M, R, HT, P, HT, P)   # [m, r, kt, pk, ht, ph]
    FTv[:, :, :, :, :8, :] = main.transpose(1, 4, 0, 3, 2, 5)
    bias = Ff[:, :, 0, :].reshape(M, R, HT, P)           # [m, r, ht, ph]
    FTv[:, :, :, 0, 8, :] = bias.transpose(1, 2, 0, 3)
    ones_row = np.ones((R, HT, M, P), np.float32)
    ones_row[:, :, AUDIO, :] = rank_w[:, None, None]
    FTv[:, :, :, 1, 8, :] = ones_row
    # reorder M to the kernel's chain order (audio last)
    FTv = np.ascontiguousarray(FTv[:, :, [0, 2, 3, 1]]).astype(bfnp)

    shared = dict(
        WGO=WGOv.astype(bfnp), WGA=WGAv.astype(bfnp),
        C1=C1v,
        W2=W2v.astype(bfnp), CB=CBv, SC=SCv,
        A2OT=A2OTv, O2AT=O2ATv, OUTWT=OUTWTv, LNV=LNVv, FT=FTv,
    )

    in_maps = []
    for c in range(NCORES):
        sl = slice(c * BS, (c + 1) * BS)
        tokTv = np.ascontiguousarray(tokens[sl].transpose(1, 2, 0)).astype(bfnp)
        u8v = np.zeros((4, BS), np.uint8)
        u8v[0:3] = pv[sl].T > 0
        u8v[3] = aum[sl] > 0
        f16v = np.zeros((7, BS), np.float32)
        f16v[0:3] = mo[sl].T
        f16v[3] = ma[sl]
        f16v[4:7] = (mo[sl] * winv[sl, None]).T
        uvv = np.zeros((M, 2, BS), np.float32)
        uvv[:, 0, :] = mask_f[sl].T
        uvv[:, 1, :] = 1.0 - mask_f[sl].T
        in_maps.append(dict(
            tokT=tokTv, u8rows=u8v, f16rows=f16v.astype(bfnp),
            uv=uvv.astype(bfnp), **shared,
        ))
    return in_maps


def kernel(**inputs):
    global _cached_nc, LAST_RESULTS
    if _cached_nc is None:
        _cached_nc = _build()
    in_maps = _host_prep(inputs)
    res = run_bass_kernel_spmd(
        _cached_nc, in_maps, core_ids=list(range(NCORES)), trace=TRACE
    )
    LAST_RESULTS = res
    out = np.stack([np.asarray(res.results[c]["outT"], np.float32).T for c in range(NCORES)], axis=0)
    return np.ascontiguousarray(out.reshape(B, H)).astype(np.float32)


# revision 30
# speedup vs baseline: 2.1121x; 1.0091x over previous
"""TRN2 Bass kernel for nn_BlendEmoBackbone: gated audio mixer + low-rank
multiplicative fusion, data-parallel over batch on 8 NeuronCores.

Strategy (v2, bf16):
- Pure data parallel: each core handles B/8 = 512 batch rows; gate MLP
  weights and LMF factor tensors replicated (bf16 halves HBM traffic).
- All activations in transposed [feature, batch] layout; every matmul
  contracts over the partition dim. bf16 operands stream 1 cycle/row on
  the PE (fp32/f32r streams at ~2 cycles/row on real TRN2).
- LayerNorm stats via PE ones-matmul column sums; -mu folded into gate
  matmuls as an extra K=1 row.
- LMF where(mask, z, 1) + x_aug ones-column folded into a K=2 tail tile
  in the same psum chain; rank_w folded into the audio factor slices.
- Factors stored partition-major [R,HT,M,P,9,P] so each (r,ht) loads
  with ONE contiguous-per-partition DMA (2.3KB lines).
- WGO gate weights resident in SBUF (single DMA, reused by 3 gates);
  the audio-source half of the other-gate matmul computed once (S).
- Row->tile broadcasts on gpsimd (partition_broadcast); abs/gelu/
  sigmoid/psum-copies on the scalar engine; products/blends on DVE in
  bf16 where precision allows.
"""

import numpy as np
import ml_dtypes
from contextlib import ExitStack

import concourse.bass as bass
from concourse import bacc
import concourse.tile as tile
from concourse import mybir
from concourse.bass_utils import run_bass_kernel_spmd

B, M, H, R = 4096, 4, 1024, 10
NCORES = 8
BS = B // NCORES          # 512 batch rows per core
MID = 512
P = 128
HT = H // P               # 8 h-tiles
MT = MID // P             # 4 mid-tiles
D3 = 3 * H
OTHERS = (0, 2, 3)
AUDIO = 1
EPS = 1e-5

f32 = mybir.dt.float32
f32r = mybir.dt.float32r
bf16 = mybir.dt.bfloat16
u8 = mybir.dt.uint8
AF = mybir.ActivationFunctionType
OP = mybir.AluOpType
bfnp = ml_dtypes.bfloat16

TRACE = False
LAST_RESULTS = None

_cached_nc = None


def _build():
    nc = bacc.Bacc("TRN2", target_bir_lowering=False, debug=False)

    # ---- DRAM parameters (per core) ----
    tokT = nc.declare_dram_parameter("tokT", [M, H, BS], bf16, isOutput=False)
    # u8 rows: 0-2 pv_j, 3 am(aum)
    u8rows = nc.declare_dram_parameter("u8rows", [4, BS], u8, isOutput=False)
    # bf16 rows: 0-2 mo_j, 3 ma, 4-6 cm_j
    f16rows = nc.declare_dram_parameter("f16rows", [7, BS], bf16, isOutput=False)
    uv = nc.declare_dram_parameter("uv", [M, 2, BS], bf16, isOutput=False)
    WGO = nc.declare_dram_parameter("WGO", [3 * HT, P, MID], bf16, isOutput=False)
    WGA = nc.declare_dram_parameter("WGA", [3 * HT, P, MID], bf16, isOutput=False)
    C1 = nc.declare_dram_parameter("C1", [P, MT, 2], f32, isOutput=False)
    W2 = nc.declare_dram_parameter("W2", [P, MT, 2], bf16, isOutput=False)
    CB = nc.declare_dram_parameter("CB", [P, 8], f32, isOutput=False)
    SC = nc.declare_dram_parameter("SC", [1, 8], f32, isOutput=False)
    # [ht_out, kt, P, P] tiled weight blocks (lhsT layout)
    A2OT = nc.declare_dram_parameter("A2OT", [HT, HT, P, P], bf16, isOutput=False)
    O2AT = nc.declare_dram_parameter("O2AT", [HT, HT, P, P], bf16, isOutput=False)
    OUTWT = nc.declare_dram_parameter("OUTWT", [HT, HT, P, P], bf16, isOutput=False)
    # cols: ln_o_w 0:8, ln_o_b 8:16, ln_a_w 16:24, ln_a_b 24:32,
    #       ln1w 32:40, ln1b 40:48, ln2w 48:56, ln2b 56:64, outb 64:72, lmfb 72:80
    LNV = nc.declare_dram_parameter("LNV", [P, 80], f32, isOutput=False)
    # partition-major factor blocks; [.., p, 0:8, :] = main k-tiles,
    # [.., 0:2, 8, :] = [bias_row; ones_or_rankw_row]
    FT = nc.declare_dram_parameter("FT", [R, HT, M, P, 9, P], bf16, isOutput=False)
    OUT = nc.declare_dram_parameter("outT", [H, BS], bf16, isOutput=True)

    with tile.TileContext(nc) as tc, ExitStack() as ctx:
        kp = ctx.enter_context(tc.tile_pool(name="konst", bufs=1))
        tokp = ctx.enter_context(tc.tile_pool(name="tokp", bufs=1))
        big = ctx.enter_context(tc.tile_pool(name="big", bufs=1))
        wk = ctx.enter_context(tc.tile_pool(name="wk", bufs=2))
        bcp = ctx.enter_context(tc.tile_pool(name="bcp", bufs=1))
        sqp = ctx.enter_context(tc.tile_pool(name="sqp", bufs=9))
        sqf = ctx.enter_context(tc.tile_pool(name="sqf", bufs=3))
        wgp = ctx.enter_context(tc.tile_pool(name="wgp", bufs=2))
        ftp = ctx.enter_context(tc.tile_pool(name="ftp", bufs=2))
        rowp = ctx.enter_context(tc.tile_pool(name="rowp", bufs=1))
        ppz = ctx.enter_context(tc.tile_pool(name="ppz", bufs=2, space="PSUM"))
        ppg = ctx.enter_context(tc.tile_pool(name="ppg", bufs=4, space="PSUM"))
        pps = ctx.enter_context(tc.tile_pool(name="pps", bufs=1, space="PSUM"))

        # ---- constants / small loads ----
        ones_k = kp.tile([P, 1], bf16)
        nc.vector.memset(ones_k, 1.0)
        ones_kf32 = kp.tile([P, 1], f32)
        nc.vector.memset(ones_kf32, 1.0)
        ones_kf = ones_kf32.bitcast(f32r)

        def bc_row_dma(dst, src_ap):
            nc.sync.dma_start(
                out=dst,
                in_=bass.AP(
                    tensor=src_ap.tensor, offset=src_ap.offset, ap=[[0, P], [1, BS]]
                ),
            )

        u8t = []
        for i in range(4):
            t = kp.tile([P, BS], u8, tag=f"u8_{i}")
            bc_row_dma(t, u8rows.ap()[i : i + 1, :])
            u8t.append(t)
        pv_t, am_t = u8t[0:3], u8t[3]
        f16t = []
        for i in range(7):
            t = kp.tile([P, BS], bf16, tag=f"f16_{i}")
            bc_row_dma(t, f16rows.ap()[i : i + 1, :])
            f16t.append(t)
        mo_t, ma_t, cm_t = f16t[0:3], f16t[3], f16t[4:7]
        uvt = []
        for m in range(M):
            t = kp.tile([2, BS], bf16, tag=f"uv_{m}")
            nc.sync.dma_start(out=t, in_=uv.ap()[m])
            uvt.append(t)
        cbt = kp.tile([P, 8], f32)
        nc.sync.dma_start(out=cbt, in_=CB.ap())
        sct = kp.tile([1, 8], f32)
        nc.sync.dma_start(out=sct, in_=SC.ap())
        lnv = kp.tile([P, 80], f32)
        nc.sync.dma_start(out=lnv, in_=LNV.ap())
        w2t = kp.tile([P, MT, 2], bf16)
        nc.sync.dma_start(out=w2t, in_=W2.ap())
        c1t = kp.tile([P, MT, 2], f32)
        nc.sync.dma_start(out=c1t, in_=C1.ap())



        # ---- tokens (transposed, bf16) ----
        tok = tokp.tile([P, M, HT, BS], bf16)
        for m in range(M):
            nc.sync.dma_start(
                out=tok[:, m], in_=tokT.ap()[m].rearrange("(ht p) b -> p ht b", p=P)
            )

        def tk(m, kt):
            return tok[:, m, kt, :]

        def tkw(m):  # whole-token [P, HT, BS] view
            return tok[:, m]

        def flat(t3):
            return t3.rearrange("p a b -> p (a b)")

        def b3(t2):  # [P,BS] -> broadcast [P,HT,BS]
            return t2.unsqueeze(1).broadcast_to([P, HT, BS])

        # ---- helpers ----
        def ln_rows(statA, statB, n, tag, hi_mu=False, par=0):
            mdt = f32 if hi_mu else bf16
            mtag = "negmuf" if hi_mu else f"negmu{par}"
            negmu = rowp.tile([1, BS], mdt, tag=mtag, name=f"negmu_{tag}")
            nc.scalar.activation(negmu, statA, AF.Copy, bias=0.0, scale=-1.0 / n)
            ex2 = rowp.tile([1, BS], f32, tag="ex2", name=f"ex2_{tag}")
            nc.scalar.activation(ex2, statB, AF.Copy, bias=0.0, scale=1.0 / n)
            msq = rowp.tile([1, BS], f32, tag="msq", name=f"msq_{tag}")
            nc.scalar.activation(msq, negmu, AF.Square)
            nc.vector.tensor_sub(ex2, ex2, msq)                      # var in place
            nc.vector.tensor_scalar_max(ex2, ex2, 0.0)               # bf16 rounding guard
            nc.scalar.activation(msq, ex2, AF.Sqrt, bias=sct[0:1, 2:3], scale=1.0)
            nc.vector.reciprocal(ex2, msq)
            rinvb = rowp.tile([1, BS], bf16, tag=f"rinvb{par}", name=f"rinvb_{tag}")
            nc.scalar.activation(rinvb, ex2, AF.Copy)
            return negmu, rinvb

        def bcast(row, tag, dt=bf16):
            """Broadcast a [1,BS] row to [P,BS] via gpsimd."""
            sb = bcp.tile([P, BS], dt, tag=f"bc_{tag}")
            nc.gpsimd.partition_broadcast(sb, row)
            return sb

        def colsum_sq(statB, tiles3, dt=bf16, name="sq"):
            """statB += per-column sums of squares of all kt slices."""
            chunks = []
            for t3 in tiles3:
                for kt in range(HT):
                    chunks.append(t3[:, kt, :])
            n = len(chunks)
            lhs = ones_k if dt == bf16 else ones_kf
            for i, ch in enumerate(chunks):
                sq = sqp.tile([P, BS], dt, tag=f"sq_{name}")
                nc.vector.tensor_mul(sq, ch, ch)
                nc.tensor.matmul(statB, lhs, sq, start=(i == 0), stop=(i == n - 1))

        def colsum(statA, tiles3, dt=bf16):
            chunks = []
            for t3 in tiles3:
                for kt in range(HT):
                    chunks.append(t3[:, kt, :])
            n = len(chunks)
            lhs = ones_k if dt == bf16 else ones_kf
            for i, ch in enumerate(chunks):
                nc.tensor.matmul(statA, lhs, ch, start=(i == 0), stop=(i == n - 1))

        def linmap(WT, src3, dst3):
            """dst3[ho] = sum_kt WT[ho,kt].T @ src3[kt]; WT streamed from DRAM."""
            for ho in range(HT):
                wt = wgp.tile([P, HT, P], bf16, tag="lin")
                nc.sync.dma_start(out=wt, in_=WT.ap()[ho].rearrange("k p c -> p k c"))
                ps = ppz.tile([P, BS], f32, tag="z")
                for kt in range(HT):
                    nc.tensor.matmul(
                        ps, wt[:, kt, :], src3[:, kt, :],
                        start=(kt == 0), stop=(kt == HT - 1),
                    )
                nc.scalar.activation(dst3[:, ho, :], ps, AF.Copy)

        # ---- a2o = audio @ a2o_w.T, in T layout (bf16) ----
        a2or = big.tile([P, HT, BS], bf16, tag="axr")
        linmap(A2OT, tkw(AUDIO), a2or)

        # S_mt = sum_k Wgo_s[k].T @ audio  (shared source half of gate1)
        S = big.tile([P, MT, BS], bf16, tag="Sg")
        for mt in range(MT):
            wS = wgp.tile([P, HT, P], bf16, tag="lin", name=f"wS{mt}")
            nc.sync.dma_start(
                out=wS,
                in_=WGO.ap()[HT : 2 * HT, :, mt * P : (mt + 1) * P].rearrange(
                    "k p c -> p k c"
                ),
            )
            ps = ppz.tile([P, BS], f32, tag="z")
            for k in range(HT):
                nc.tensor.matmul(
                    ps, wS[:, k, :], tk(AUDIO, k),
                    start=(k == 0), stop=(k == HT - 1),
                )
            nc.scalar.activation(S[:, mt, :], ps, AF.Copy)

        omt = big.tile([P, HT, BS], bf16, tag="om")  # others_mean accumulator
        mix_src = {"x": a2or}

        def gate_phase1(j, mj):
            """Stats + gate1 matmuls + LN rows for gate j. Emission order
            keeps the PE fed: gate1 halves (no stats dependency) are
            interleaved with the DVE-paced stat chains."""
            is_audio = j == 3
            t_m = AUDIO if is_audio else mj
            t3 = tkw(t_m)
            s3 = omt if is_audio else tkw(AUDIO)

            abs3 = big.tile([P, HT, BS], bf16, tag="abs", name=f"abs3_{j}")
            statA = pps.tile([1, BS], f32, tag="statA", name=f"statA_{j}")
            statB = pps.tile([1, BS], f32, tag="statB", name=f"statB_{j}")
            gps = [
                ppg.tile([P, BS], f32, tag="g", name=f"gps{j}_{mt}")
                for mt in range(MT)
            ]
            if is_audio:
                parts = [(WGA, 0, t3), (WGA, 1, s3), (WGA, 2, abs3)]
            else:
                parts = [(WGO, 0, t3), (WGO, 2, abs3)]

            # DVE production first: d, |d| (ACT), d^2 tiles
            dsq = []
            for k in range(HT):
                dk = wk.tile([P, BS], bf16, tag="dk")
                nc.vector.tensor_sub(dk, t3[:, k, :], s3[:, k, :])
                nc.scalar.activation(abs3[:, k, :], dk, AF.Abs)
                sq = sqp.tile([P, BS], bf16, tag="sq_sq")
                nc.vector.tensor_mul(sq, dk, dk)
                dsq.append(sq)

            def mt_chain(mt):
                for pi, (WG, part, rhs3) in enumerate(parts):
                    w = wgp.tile([P, HT, P], bf16, tag="lin", name=f"wg{j}_{mt}_{part}")
                    nc.sync.dma_start(
                        out=w,
                        in_=WG.ap()[
                            part * HT : (part + 1) * HT, :, mt * P : (mt + 1) * P
                        ].rearrange("k p c -> p k c"),
                    )
                    for k in range(HT):
                        nc.tensor.matmul(
                            gps[mt], w[:, k, :], rhs3[:, k, :],
                            start=(pi == 0 and k == 0),
                            stop=(pi == len(parts) - 1 and k == HT - 1),
                        )

            mt_chain(0)
            mt_chain(1)
            for i, sq in enumerate(dsq):
                nc.tensor.matmul(statB, ones_k, sq, start=(i == 0), stop=False)
            for t_src in (t3, s3):
                for k in range(HT):
                    sq = sqp.tile([P, BS], bf16, tag="sq_sq")
                    nc.vector.tensor_mul(sq, t_src[:, k, :], t_src[:, k, :])
                    nc.tensor.matmul(
                        statB, ones_k, sq,
                        start=False, stop=(t_src is s3 and k == HT - 1),
                    )
            mt_chain(2)
            colsum(statA, [t3, s3, abs3])
            mt_chain(3)
            negmu, rinvb = ln_rows(statA, statB, D3, f"g{j}", par=j % 2)
            return abs3, negmu, rinvb, gps

        def gate_phase2a(j, mj, abs3, negmu, rinvb, gps):
            """Gate layer 2 + mix pre-activation for gate j."""
            is_audio = j == 3
            t3 = tkw(AUDIO if is_audio else mj)
            rb = bcast(rinvb, "rb")
            nmb = bcast(negmu, "nm")
            cb_off = 4 if is_audio else 0
            col = 1 if is_audio else 0
            gp = pps.tile([1, BS], f32, tag="statA", name=f"gp{j}")
            for mt in range(MT):
                hm = wk.tile([P, BS], f32, tag="hm")
                # hm = gps + (-mu)*c1 [+ S]; then * rinv
                nc.vector.scalar_tensor_tensor(
                    hm, nmb, c1t[:, mt, col : col + 1], gps[mt],
                    op0=OP.mult, op1=OP.add,
                )
                if not is_audio:
                    nc.vector.tensor_add(hm, hm, S[:, mt, :])
                nc.vector.tensor_mul(hm, hm, rb)
                hg1 = wk.tile([P, BS], bf16, tag="hg", name=f"hg{mt}")
                nc.scalar.activation(
                    hg1, hm, AF.Gelu,
                    bias=cbt[:, cb_off + mt : cb_off + mt + 1], scale=1.0,
                )
                nc.tensor.matmul(
                    gp, w2t[:, mt, col : col + 1], hg1,
                    start=(mt == 0), stop=(mt == MT - 1),
                )
            g_row = rowp.tile([1, BS], bf16, tag="g_row")
            nc.scalar.activation(
                g_row, gp, AF.Sigmoid, bias=sct[0:1, col : col + 1], scale=1.0,
            )
            gb = bcast(g_row, "gb")
            # pre = t + g * (a2o | o2a)
            src = mix_src["x"]
            pre = big.tile([P, HT, BS], bf16, tag=f"pre{j % 2}", name=f"pre{j}")
            nc.vector.tensor_mul(pre, src, b3(gb))
            nc.vector.tensor_add(flat(pre), flat(pre), flat(t3))
            return pre

        def gate_phase2b(j, mj, pre):
            """Mix LN + blend for gate j."""
            is_audio = j == 3
            t_m = AUDIO if is_audio else mj
            t3 = tkw(t_m)
            stat2A = pps.tile([1, BS], f32, tag="statA", name=f"stat2A_{j}")
            stat2B = pps.tile([1, BS], f32, tag="statB", name=f"stat2B_{j}")
            colsum(stat2A, [pre])
            colsum_sq(stat2B, [pre])
            negmu2, rinvb2 = ln_rows(stat2A, stat2B, H, f"u{j}", par=2 + (j % 2))
            mb = bcast(negmu2, "mb")
            rb2 = bcast(rinvb2, "rb2")
            wcol = 16 if is_audio else 0
            bcol = 24 if is_audio else 8
            sm = am_t if is_audio else pv_t[j]
            bmf = ma_t if is_audio else mo_t[j]
            # whole-token LN apply + blend: tok = bmf * (sm ? ln(pre) : t)
            q3 = big.tile([P, HT, BS], bf16, tag="q3", name=f"q3_{j}")
            nc.vector.tensor_add(q3, pre, b3(mb))
            nc.vector.tensor_mul(q3, q3, b3(rb2))
            for kt in range(HT):
                nc.vector.tensor_scalar(
                    q3[:, kt, :], q3[:, kt, :],
                    lnv[:, wcol + kt : wcol + kt + 1], lnv[:, bcol + kt : bcol + kt + 1],
                    op0=OP.mult, op1=OP.add,
                )
            nc.vector.copy_predicated(t3, b3(sm), q3)
            nc.vector.tensor_mul(t3, t3, b3(bmf))
            if not is_audio:
                if j == 0:
                    nc.vector.tensor_mul(omt, t3, b3(cm_t[j]))
                else:
                    tmp3 = big.tile([P, HT, BS], bf16, tag="q3", name=f"omtmp_{j}")
                    nc.vector.tensor_mul(tmp3, t3, b3(cm_t[j]))
                    nc.vector.tensor_add(omt, omt, tmp3)

        # LMF chain-index order: factors stored with M reordered as MORD so
        # the audio (blended last) chain comes last; r=0 partial products
        # for the non-audio tokens are emitted between mixer phases to keep
        # the PE busy during the gates' serial post-chains.
        MORD = (0, 2, 3, 1)
        uvt_l = [uvt[m] for m in MORD]
        acc = big.tile([P, HT, BS], f32r, tag="acc")
        soth = big.tile([P, HT, BS], bf16, tag="soth")

        def lmf_part_chain(r, mi, dst):
            """Partial-product chains for rank r emitted out of band: dst
            accumulates z products for chain mi; mi==3 folds into acc."""
            for ht in range(HT):
                ft = ftp.tile([P, 9, P], bf16, tag="ft0")
                nc.sync.dma_start(out=ft, in_=FT.ap()[r, ht, mi])
                zp = ppz.tile([P, BS], f32, tag="z")
                for kt in range(HT):
                    nc.tensor.matmul(
                        zp, ft[:, kt, :], tk(MORD[mi], kt), start=(kt == 0), stop=False
                    )
                nc.tensor.matmul(zp, ft[0:2, 8, :], uvt_l[mi], start=False, stop=True)
                if mi == 0:
                    nc.scalar.activation(dst[:, ht, :], zp, AF.Copy)
                elif mi < 3:
                    nc.vector.tensor_mul(dst[:, ht, :], dst[:, ht, :], zp)
                elif r == 0:
                    nc.vector.tensor_mul(acc[:, ht, :], dst[:, ht, :], zp)
                else:
                    s0 = wk.tile([P, BS], f32, tag="s0")
                    nc.vector.tensor_mul(s0, dst[:, ht, :], zp)
                    nc.vector.tensor_add(acc[:, ht, :], acc[:, ht, :], s0)

        # software-pipelined emission: gate j+1's stats+gate1 overlap gate
        # j's post-matmul chain on the PE; LMF r=0 chains fill blend windows.
        p1, p2 = {}, {}
        p1[0] = gate_phase1(0, OTHERS[0])
        p1[1] = gate_phase1(1, OTHERS[1])
        p2[0] = gate_phase2a(0, OTHERS[0], *p1[0])
        p1[2] = gate_phase1(2, OTHERS[2])
        p2[1] = gate_phase2a(1, OTHERS[1], *p1[1])
        gate_phase2b(0, OTHERS[0], p2[0])
        p2[2] = gate_phase2a(2, OTHERS[2], *p1[2])
        gate_phase2b(1, OTHERS[1], p2[1])
        gate_phase2b(2, OTHERS[2], p2[2])
        lmf_part_chain(0, 0, soth)
        lmf_part_chain(0, 1, soth)

        # ---- o2a = others_mean @ o2a_w.T ----
        o2ar = big.tile([P, HT, BS], bf16, tag="axr", name="o2ar")
        linmap(O2AT, omt, o2ar)
        mix_src["x"] = o2ar

        p1[3] = gate_phase1(3, AUDIO)
        lmf_part_chain(0, 2, soth)
        p2[3] = gate_phase2a(3, AUDIO, *p1[3])
        # r=1 partial products fill the audio post-chain/blend window
        soth1 = big.tile([P, HT, BS], bf16, tag="axr", name="soth1")
        lmf_part_chain(1, 0, soth1)
        gate_phase2b(3, AUDIO, p2[3])
        lmf_part_chain(1, 1, soth1)
        lmf_part_chain(1, 2, soth1)
        lmf_part_chain(0, 3, soth)
        lmf_part_chain(1, 3, soth1)

        # ---- LMF ranks 2..R-1; LN1 stats interleaved into the last rank ----
        stat3A = pps.tile([1, BS], f32, tag="statA", name="stat3A")
        stat3B = pps.tile([1, BS], f32, tag="statB", name="stat3B")

        def stat3_for(ht, start, stop):
            nc.vector.tensor_scalar_add(
                acc[:, ht, :], acc[:, ht, :], lnv[:, 72 + ht : 72 + ht + 1]
            )
            sq = sqf.tile([P, BS], f32r, tag="sq_f")
            nc.vector.tensor_mul(sq, acc[:, ht, :], acc[:, ht, :])
            nc.tensor.matmul(stat3A, ones_kf, acc[:, ht, :], start=start, stop=stop)
            nc.tensor.matmul(stat3B, ones_kf, sq, start=start, stop=stop)

        for r in range(2, R):
            last = r == R - 1
            for ht in range(HT):
                ft = ftp.tile([P, M, 9, P], bf16, tag="ft")
                nc.sync.dma_start(
                    out=ft, in_=FT.ap()[r, ht].rearrange("m p k c -> p m k c")
                )
                zps = []
                for m in range(M):
                    zp = ppz.tile([P, BS], f32, tag="z")
                    for kt in range(HT):
                        nc.tensor.matmul(
                            zp, ft[:, m, kt, :], tk(MORD[m], kt),
                            start=(kt == 0), stop=False,
                        )
                    nc.tensor.matmul(zp, ft[0:2, m, 8, :], uvt_l[m], start=False, stop=True)
                    zps.append(zp)
                s0 = wk.tile([P, BS], f32, tag="s0")
                nc.scalar.activation(s0, zps[0], AF.Copy)
                nc.vector.tensor_mul(s0, s0, zps[1])
                nc.vector.tensor_mul(s0, s0, zps[2])
                nc.vector.tensor_mul(s0, s0, zps[3])
                nc.vector.tensor_add(acc[:, ht, :], acc[:, ht, :], s0)
                if last and ht >= 1:
                    stat3_for(ht - 1, start=(ht == 1), stop=False)
        stat3_for(HT - 1, start=False, stop=True)
        negmu3, rinvb3 = ln_rows(stat3A, stat3B, H, "l1", hi_mu=True)
        mb3 = bcast(negmu3, "mbf", dt=f32)
        rb3 = bcast(rinvb3, "rb2")
        h1 = big.tile([P, HT, BS], bf16, tag="pre0", name="h1")
        nc.vector.tensor_add(h1, acc, b3(mb3))
        nc.vector.tensor_mul(h1, h1, b3(rb3))
        for kt in range(HT):
            nc.vector.tensor_scalar(
                h1[:, kt, :], h1[:, kt, :],
                lnv[:, 32 + kt : 32 + kt + 1], lnv[:, 40 + kt : 40 + kt + 1],
                op0=OP.mult, op1=OP.add,
            )

        # h2 = gelu(h1 @ out_w.T + out_b); LN2; write out
        h2 = big.tile([P, HT, BS], bf16, tag="abs", name="h2")
        for ho in range(HT):
            wt = wgp.tile([P, HT, P], bf16, tag="lin", name=f"ow{ho}")
            nc.sync.dma_start(out=wt, in_=OUTWT.ap()[ho].rearrange("k p c -> p k c"))
            ps = ppz.tile([P, BS], f32, tag="z")
            for kt in range(HT):
                nc.tensor.matmul(
                    ps, wt[:, kt, :], h1[:, kt, :],
                    start=(kt == 0), stop=(kt == HT - 1),
                )
            nc.scalar.activation(
                h2[:, ho, :], ps, AF.Gelu, bias=lnv[:, 64 + ho : 64 + ho + 1], scale=1.0
            )
        stat4A = pps.tile([1, BS], f32, tag="statA", name="stat4A")
        stat4B = pps.tile([1, BS], f32, tag="statB", name="stat4B")
        colsum(stat4A, [h2])
        colsum_sq(stat4B, [h2])
        negmu4, rinvb4 = ln_rows(stat4A, stat4B, H, "l2")
        mb4 = bcast(negmu4, "mb")
        rb4 = bcast(rinvb4, "rb2")
        fin3 = big.tile([P, HT, BS], bf16, tag="q3", name="fin3")
        nc.vector.tensor_add(fin3, h2, b3(mb4))
        nc.vector.tensor_mul(fin3, fin3, b3(rb4))
        for kt in range(HT):
            nc.vector.tensor_scalar(
                fin3[:, kt, :], fin3[:, kt, :],
                lnv[:, 48 + kt : 48 + kt + 1], lnv[:, 56 + kt : 56 + kt + 1],
                op0=OP.mult, op1=OP.add,
            )
            nc.sync.dma_start(out=OUT.ap()[kt * P : (kt + 1) * P, :], in_=fin3[:, kt, :])

    nc.compile()
    return nc


def _host_prep(inputs):
    tokens = np.asarray(inputs["tokens"], np.float32)
    token_mask = np.asarray(inputs["token_mask"])
    mask_f = token_mask.astype(np.float32)

    mo = mask_f[:, list(OTHERS)]                      # [B,3]
    ma = mask_f[:, AUDIO]                             # [B]
    pv = mo * ma[:, None]                             # [B,3]
    winv = (1.0 / np.clip(mo.sum(1), 1.0, None)).astype(np.float32)
    aum = ma * (mo.max(1) > 0)                        # [B]

    go_w1 = np.asarray(inputs["go_w1"], np.float32)
    ga_w1 = np.asarray(inputs["ga_w1"], np.float32)

    def gate_prep(w1, b1, lnw, lnb):
        W1w = w1 * lnw[None, :]                       # [MID, 3H]
        c1 = np.ascontiguousarray(W1w.sum(1).reshape(1, MID))
        cb = w1 @ lnb + b1                            # [MID]
        Wblocks = np.ascontiguousarray(W1w.T).reshape(3 * HT, P, MID)
        return Wblocks, c1, cb

    WGOv, c1go, cbgo = gate_prep(
        go_w1, np.asarray(inputs["go_b1"], np.float32),
        np.asarray(inputs["ln_go_w"], np.float32), np.asarray(inputs["ln_go_b"], np.float32),
    )
    WGAv, c1ga, cbga = gate_prep(
        ga_w1, np.asarray(inputs["ga_b1"], np.float32),
        np.asarray(inputs["ln_ga_w"], np.float32), np.asarray(inputs["ln_ga_b"], np.float32),
    )
    CBv = np.ascontiguousarray(
        np.concatenate([cbgo.reshape(MT, P).T, cbga.reshape(MT, P).T], axis=1)
    ).astype(np.float32)                              # [P, 8]
    W2v = np.stack(
        [np.asarray(inputs["go_w2"], np.float32).reshape(MID),
         np.asarray(inputs["ga_w2"], np.float32).reshape(MID)], axis=1
    )                                                 # [MID, 2]
    W2v = np.ascontiguousarray(W2v.reshape(MT, P, 2).transpose(1, 0, 2))
    C1v = np.stack([c1go.reshape(MID), c1ga.reshape(MID)], axis=1)
    C1v = np.ascontiguousarray(C1v.reshape(MT, P, 2).transpose(1, 0, 2)).astype(np.float32)
    SCv = np.zeros((1, 8), np.float32)
    SCv[0, 0] = np.asarray(inputs["go_b2"], np.float32).reshape(-1)[0]
    SCv[0, 1] = np.asarray(inputs["ga_b2"], np.float32).reshape(-1)[0]
    SCv[0, 2] = EPS

    def tile_blocks(w):
        wt = np.ascontiguousarray(np.asarray(w, np.float32).T)    # [H_in, H_out]
        return np.ascontiguousarray(
            wt.reshape(HT, P, HT, P).transpose(2, 0, 1, 3)
        ).astype(bfnp)

    A2OTv = tile_blocks(inputs["a2o_w"])
    O2ATv = tile_blocks(inputs["o2a_w"])
    OUTWTv = tile_blocks(inputs["out_w"])

    def cols(name):
        return np.asarray(inputs[name], np.float32).reshape(HT, P).T

    LNVv = np.zeros((P, 80), np.float32)
    for i, name in enumerate(
        ["ln_o_w", "ln_o_b", "ln_a_w", "ln_a_b", "out_ln1_w", "out_ln1_b",
         "out_ln2_w", "out_ln2_b", "out_b", "lmf_bias"]
    ):
        LNVv[:, 8 * i : 8 * (i + 1)] = cols(name)

    factors = np.asarray(inputs["factors"], np.float32)
    rank_w = np.asarray(inputs["rank_w"], np.float32)
    Ff = factors.copy()
    Ff[AUDIO] = Ff[AUDIO] * rank_w[:, None, None]
    # partition-major layout [R, HT, M, P, 9, P]
    FTv = np.zeros((R, HT, M, P, 9, P), np.float32)
    main = Ff[:, :, 1:, :].reshape(M, R, HT, P, HT, P)   # [m, r, kt, pk, ht, ph]
    FTv[:, :, :, :, :8, :] = main.transpose(1, 4, 0, 3, 2, 5)
    bias = Ff[:, :, 0, :].reshape(M, R, HT, P)           # [m, r, ht, ph]
    FTv[:, :, :, 0, 8, :] = bias.transpose(1, 2, 0, 3)
    ones_row = np.ones((R, HT, M, P), np.float32)
    ones_row[:, :, AUDIO, :] = rank_w[:, None, None]
    FTv[:, :, :, 1, 8, :] = ones_row
    # reorder M to the kernel's chain order (audio last)
    FTv = np.ascontiguousarray(FTv[:, :, [0, 2, 3, 1]]).astype(bfnp)

    shared = dict(
        WGO=WGOv.astype(bfnp), WGA=WGAv.astype(bfnp),
        C1=C1v,
        W2=W2v.astype(bfnp), CB=CBv, SC=SCv,
        A2OT=A2OTv, O2AT=O2ATv, OUTWT=OUTWTv, LNV=LNVv, FT=FTv,
    )

    in_maps = []
    for c in range(NCORES):
        sl = slice(c * BS, (c + 1) * BS)
        tokTv = np.ascontiguousarray(tokens[sl].transpose(1, 2, 0)).astype(bfnp)
        u8v = np.zeros((4, BS), np.uint8)
        u8v[0:3] = pv[sl].T > 0
        u8v[3] = aum[sl] > 0
        f16v = np.zeros((7, BS), np.float32)
        f16v[0:3] = mo[sl].T
        f16v[3] = ma[sl]
        f16v[4:7] = (mo[sl] * winv[sl, None]).T
        uvv = np.zeros((M, 2, BS), np.float32)
        uvv[:, 0, :] = mask_f[sl].T
        uvv[:, 1, :] = 1.0 - mask_f[sl].T
        in_maps.append(dict(
            tokT=tokTv, u8rows=u8v, f16rows=f16v.astype(bfnp),
            uv=uvv.astype(bfnp), **shared,
        ))
    return in_maps


def kernel(**inputs):
    global _cached_nc, LAST_RESULTS
    if _cached_nc is None:
        _cached_nc = _build()
    in_maps = _host_prep(inputs)
    res = run_bass_kernel_spmd(
        _cached_nc, in_maps, core_ids=list(range(NCORES)), trace=TRACE
    )
    LAST_RESULTS = res
    out = np.stack([np.asarray(res.results[c]["outT"], np.float32).T for c in range(NCORES)], axis=0)
    return np.ascontiguousarray(out.reshape(B, H)).astype(np.float32)
